# revision 2
# baseline (speedup 1.0000x reference)
"""Trainium2 Bass kernel for nn_Block_78993038508729 (dense transformer
block: rmsnorm -> causal MHA (degenerate rope cancels) -> rmsnorm ->
top-2 MoE with SwiGLU experts).

Two launches on 8 cores; host does the O(T*D) glue between them.

Launch A (attention): tensor-parallel over heads, 2 heads/core.
  q/k/v projections run as two-term compensated fp8e4m3 DoubleRow
  matmuls (main + residual operands, power-of-2 scaled); the scale
  falls out through the exp() scale and the softmax-denominator ones
  column, so no descale ops exist anywhere.  Scores/AV/out-proj stay
  bf16 (fp8 there perturbs x2 enough to flip top-2 routing picks).
  The causal mask is ADDED into the scores psum by a mask @ identity
  matmul before exp, so masking costs ~53ns of PE per diagonal block
  instead of a Pool multiply after exp.  Softmax denominators ride the
  AV matmul as a ones column scaled 2^16 (cancels the operand scales);
  normalization is reciprocal (DVE) + ones-outer-product broadcast
  (PE) + multiply (DVE).

Launch B (experts): expert-parallel, one expert/core, fp8 DoubleRow.
  Tokens for the up projection are pre-scaled by routing-weight *
  1/sqrt(LAYER_DEPTH) on the host, so expert outputs leave the down
  matmul already weighted.  The down projection is computed
  token-major (out[d_tile, token]) which needs 64 wide-N matmuls
  instead of 80 and no per-tile weight multiply.  All fp8 tensors are
  power-of-2 scaled into e4m3's happy range; scales cancel via the
  silu activation scale, one scalar_tensor_tensor, and the output
  copy scale.

Numerics: bf16-class noise end to end (the comp8 projections are ~2.6x
more accurate than bf16 operands); rel err vs the fp32 reference is a
few e-3, and top-2 routing matches the reference exactly on the
reference input distribution.
"""

import sys

if "/opt/trn_rl_repo" not in sys.path:
    sys.path.insert(0, "/opt/trn_rl_repo")

import math

import ml_dtypes
import numpy as np

import concourse.bass as bass
import concourse.mybir as mybir
import concourse.tile as tile
from concourse import bacc
from concourse.bass_utils import run_bass_kernel_spmd

F32 = mybir.dt.float32
BF16 = mybir.dt.bfloat16
F8 = mybir.dt.float8e4
AF = mybir.ActivationFunctionType
PM = mybir.MatmulPerfMode
BF16_NP = ml_dtypes.bfloat16
F8_NP = ml_dtypes.float8_e4m3fn

B, T, D = 1, 2048, 1024
NH, HD = 16, 64
E, K, H = 8, 2, 2048
LAYER_DEPTH = 12
EPS = 1e-8
NCORES = 8
HPC = NH // NCORES          # heads per core = 2
CW = HPC * HD               # per-core head-column width = 128
CAP = 552                   # token capacity per expert core (max load 547)
NHT = H // 128              # moe h tiles
MOE_SCALE = 1.0 / math.sqrt(LAYER_DEPTH)

# fp8 power-of-2 scales (all exact in fp arithmetic)
SH = 32.0                   # normed activations h
SW = 2048.0                 # attention weight matrices (s=0.02 scale)
SPROJ = SH * SW             # scale carried by q/k/v psums = 2^16
SGT = 32.0                  # moe gate tokens
SUT = 128.0                 # moe up tokens (incl routing weight fold)
SWG = 2048.0                # gate_w
SWU = 2048.0                # up_w
SGU = 64.0                  # g*u activations
SWD = 2048.0                # down_w
MASKV = -3.0e14             # causal mask additive value (pre-exp, scaled)

_CACHE: dict = {}
MOE_ROUNDS = 0


def _bacc(n_cores):
    return bacc.Bacc("TRN2", target_bir_lowering=False, debug=False,
                     num_devices=n_cores)


def _f8(a):
    return np.clip(a, -448, 448).astype(F8_NP)


def _comp8(a, s):
    """Two-term compensated fp8: a*s ~= main + resid."""
    m = _f8(a * s)
    r = _f8(a * s - m.astype(np.float32))
    return m, r


# --------------------------------------------------------------------------
# Launch A: attention (head-sharded; comp8 projections, bf16 attention).
# Per-core inputs:
#   hm, hr  [128, 8, T] f8    comp8 pair of rmsnorm(x)*norm1_w * 32,
#                             hm[p,c,t] = (h*32)[t, 128c+p]
#   w8 (6x) [128, 8, CW] f8   wq/wk/wv main+resid, w8[p,c,m]=W[128c+p,m]*2048
#   wo      [128, D] bf16     wo rows for this core's head columns
#   maskT   [128, 128] bf16   maskT[q,k] = MASKV if k > q else 0
#   ident   [128, 128] bf16   identity
#   onesb   [1, 128] bf16     ones row (denominator broadcast outer product)
# Output:
#   part    [T, D] bf16       this core's partial of y @ wo (normalized)
# --------------------------------------------------------------------------

def build_attn():
    nc = _bacc(NCORES)
    NC = D // 128             # 8 contraction chunks
    NC2 = NC // 2             # 4 DoubleRow chunk-pairs
    NJ = T // 512             # 4 query blocks
    hm_d = nc.dram_tensor("hm", [128, NC, T], F8, kind="ExternalInput")
    hr_d = nc.dram_tensor("hr", [128, NC, T], F8, kind="ExternalInput")
    w_d = {w: nc.dram_tensor(w, [128, NC, CW], F8, kind="ExternalInput")
           for w in ("wqm", "wqr", "wkm", "wkr", "wvm", "wvr")}
    wo_d = nc.dram_tensor("wo", [128, D], BF16, kind="ExternalInput")
    maskT_d = nc.dram_tensor("maskT", [128, 128], BF16, kind="ExternalInput")
    ident_d = nc.dram_tensor("ident", [128, 128], BF16, kind="ExternalInput")
    onesb_d = nc.dram_tensor("onesb", [1, 128], BF16, kind="ExternalInput")
    part_d = nc.dram_tensor("part", [T, D], BF16, kind="ExternalOutput")

    with tile.TileContext(nc, num_cores=NCORES) as tc:
        with (
            tc.tile_pool(name="const", bufs=1) as const,
            tc.tile_pool(name="big", bufs=1) as bigp,
            tc.tile_pool(name="et", bufs=6) as etp,
            tc.tile_pool(name="dens", bufs=4) as densp,
            tc.tile_pool(name="out", bufs=6) as outp,
            tc.tile_pool(name="ss", bufs=2, space="PSUM") as ps_s,
            tc.tile_pool(name="pa", bufs=2, space="PSUM") as ps_a,
            tc.tile_pool(name="mm", bufs=2, space="PSUM") as ps_m,
        ):
            # DMA order: the first projection chain (wqm/wkm x hm[0:1024])
            # must land first; residual operands follow right behind.
            w8 = {w: const.tile([128, NC, CW], F8, name=w)
                  for w in ("wqm", "wqr", "wkm", "wkr", "wvm", "wvr")}
            hm = bigp.tile([128, NC, T], F8)
            hr = bigp.tile([128, NC, T], F8)
            nc.sync.dma_start(out=w8["wqm"][:], in_=w_d["wqm"][:, :, :])
            nc.sync.dma_start(out=hm[:, :, 0:512], in_=hm_d[:, :, 0:512])
            nc.sync.dma_start(out=w8["wkm"][:], in_=w_d["wkm"][:, :, :])
            nc.sync.dma_start(out=w8["wqr"][:], in_=w_d["wqr"][:, :, :])
            nc.sync.dma_start(out=hr[:, :, 0:512], in_=hr_d[:, :, 0:512])
            nc.sync.dma_start(out=w8["wkr"][:], in_=w_d["wkr"][:, :, :])
            nc.sync.dma_start(out=hm[:, :, 512:1024], in_=hm_d[:, :, 512:1024])
            nc.sync.dma_start(out=hr[:, :, 512:1024], in_=hr_d[:, :, 512:1024])
            for w in ("wvm", "wvr"):
                nc.sync.dma_start(out=w8[w][:], in_=w_d[w][:, :, :])
            maskT = const.tile([128, 128], BF16)
            nc.sync.dma_start(out=maskT[:], in_=maskT_d[:, :])
            ident = const.tile([128, 128], BF16)
            nc.sync.dma_start(out=ident[:], in_=ident_d[:, :])
            onesb = const.tile([1, 128], BF16)
            nc.sync.dma_start(out=onesb[:], in_=onesb_d[:, :])
            nc.sync.dma_start(out=hm[:, :, 1024:2048], in_=hm_d[:, :, 1024:2048])
            nc.sync.dma_start(out=hr[:, :, 1024:2048], in_=hr_d[:, :, 1024:2048])
            wo = const.tile([128, D], BF16)
            nc.sync.dma_start(out=wo[:], in_=wo_d[:, :])

            # Warm the PE during the DMA lead-in (pstate ramp: full speed
            # after ~3us of continuous execution; no reset on later gaps).
            # Memset on the idle Pool engine so the train starts at t~0.
            warm = bigp.tile([128, 512], BF16)
            nc.gpsimd.memset(warm[:], 0.0)
            pwarm = ps_m.tile([128, 512], F32, tag="mm", name="pwarm")
            for _ in range(7):
                nc.tensor.matmul(pwarm[:], warm[:, 0:128], warm[:],
                                 start=True, stop=True)

            qT = bigp.tile([128, T], BF16)
            kT = bigp.tile([128, T], BF16)
            vT = bigp.tile([128, T], BF16)
            yT = bigp.tile([128, T], BF16)
            # v in [tok, hd] layout, grouped [head, 65]; the 65th column is
            # 2^16 so the denominator cancels the q/k/v operand scales.
            vdir = bigp.tile([128, T // 128, HPC, HD + 1], BF16)
            nc.vector.memset(vdir[:, :, :, HD], float(SPROJ))

            def proj(J, dst, wm, wr, interleave_with=None):
                """comp8 projection for a 1024-token superblock J into a
                [hd-cols, tok] transposed tile (q/k/v uniformly).  With
                interleave_with=(dst2, wm2, wr2) the two projections'
                chains alternate, ordered so the residual-operand chains
                (which need the trailing DMA transfers) come last."""
                jsl = bass.ts(J, 1024)
                projs = [(dst, wm, wr, ps_s.tile([128, 1024], F32, tag="ss",
                                                 name=f"pp{wm}{J}"))]
                if interleave_with is not None:
                    dst2, wm2, wr2 = interleave_with
                    projs.append((dst2, wm2, wr2,
                                  ps_s.tile([128, 1024], F32, tag="ss",
                                            name=f"pp{wm2}{J}")))
                # hf-outer so each 512-token half's chains run as soon as
                # its hm/hr DMA chunks land (matmul N is capped at 512);
                # each half is copied out right away so consumers (scores,
                # transposes) unblock while the next half computes.
                for hf in range(2):
                    hfs = slice(J * 1024 + 512 * hf, J * 1024 + 512 * (hf + 1))
                    for ci, hh_sel in enumerate(("mm", "rm", "mr")):
                        for d2, m2, r2, pq in projs:
                            wn = m2 if hh_sel[0] == "m" else r2
                            hh = hm if hh_sel[1] == "m" else hr
                            for c in range(NC2):
                                nc.tensor.matmul(
                                    pq[:, 512 * hf:512 * (hf + 1)],
                                    w8[wn][:, 2 * c:2 * c + 2, :],
                                    hh[:, 2 * c:2 * c + 2, hfs],
                                    start=(ci == 0 and c == 0),
                                    stop=(ci == 2 and c == NC2 - 1),
                                    perf_mode=PM.DoubleRow)
                    for d2, m2, r2, pq in projs:
                        nc.vector.tensor_copy(d2[:, hfs],
                                              pq[:, 512 * hf:512 * (hf + 1)])

            def v_tr(i):
                """vT token tile i -> vdir [tok, h, hd] via PE transpose."""
                pv = ps_m.tile([128, 512], F32, tag="mm")
                nc.tensor.matmul(pv[:, 0:128], vT[:, bass.ts(i, 128)],
                                 ident[:], start=True, stop=True)
                nc.vector.tensor_copy(
                    vdir[:, i, :, 0:HD],
                    pv[:, 0:128].rearrange("p (h d) -> p h d", d=HD))

            def qk_chunks(J):
                return [lambda: proj(J, qT, "wqm", "wqr",
                                     interleave_with=(kT, "wkm", "wkr"))]

            def v_chunks(J):
                out = [lambda: proj(J, vT, "wvm", "wvr")]
                out += [lambda i=i: v_tr(i) for i in range(8 * J, 8 * J + 8)]
                return out

            def outproj_chunk(i, engines=("v", "s")):
                """output projection + writeback for token tile i (two
                [128,512] psum halves on the small-matmul ring)."""
                ot = outp.tile([128, 1024], BF16, tag="ot")
                for half in range(2):
                    po = ps_m.tile([128, 512], F32, tag="mm")
                    nc.tensor.matmul(
                        po[:], yT[:, bass.ts(i, 128)],
                        wo[:, 512 * half:512 * (half + 1)],
                        start=True, stop=True)
                    dst = ot[:, 512 * half:512 * (half + 1)]
                    if engines[half] == "v":
                        nc.vector.tensor_copy(dst, po[:])
                    else:
                        nc.scalar.copy(dst, po[:])
                nc.sync.dma_start(out=part_d[bass.ts(i, 128), :], in_=ot[:])

            def outproj_chunks(j):
                return [lambda i=i: outproj_chunk(i)
                        for i in range(4 * j, 4 * j + 4)]

            paccs = {}
            ets = {}
            norm_pending = []

            def stage_scores(j, h, ib0):
                """scores + causal mask + exp for key blocks (ib0, ib0+1)."""
                jsl = bass.ts(j, 512)
                hsl = slice(h * HD, (h + 1) * HD)
                pss = ps_s.tile([128, 1024], F32, tag="ss")
                et = etp.tile([128, 1024], BF16, tag="et")
                ets[(j, h, ib0)] = et
                offs = []
                for half, ib in enumerate((ib0, ib0 + 1)):
                    off = max(0, (ib - 4 * j) * 128)
                    offs.append(off)
                    diag = ib >= 4 * j
                    nc.tensor.matmul(
                        pss[:, 512 * half + off:512 * (half + 1)],
                        kT[hsl, bass.ts(ib, 128)],
                        qT[hsl, jsl][:, off:512],
                        start=True, stop=not diag)
                    if diag:
                        # additive causal mask: psum += maskT.T @ I
                        nc.tensor.matmul(
                            pss[:, 512 * half + off:512 * half + off + 128],
                            maskT[:], ident[:], start=False, stop=True)
                nc.scalar.activation(
                    out=et[:, offs[0]:1024], in_=pss[:, offs[0]:1024],
                    func=AF.Exp, scale=1.0 / (math.sqrt(HD) * SPROJ * SPROJ))

            def stage_av(j, h, ib0):
                jsl = bass.ts(j, 512)
                nblk = 4 * j + 4
                hsl = slice(h * HD, (h + 1) * HD)
                if ib0 == 0:
                    paccs[(j, h)] = ps_a.tile([HD + 1, 512], F32,
                                              tag="pacc", name=f"pacc{j}_{h}")
                pacc = paccs[(j, h)]
                et = ets.pop((j, h, ib0))
                for half, ib in enumerate((ib0, ib0 + 1)):
                    off = max(0, (ib - 4 * j) * 128)
                    nc.tensor.matmul(
                        pacc[:, off:512], vdir[:, ib, h, :],
                        et[:, 512 * half + off:512 * (half + 1)],
                        start=(ib == 0), stop=(ib == nblk - 1))
                if ib0 + 2 >= nblk:
                    # normalize: yT = pacc[0:64] * (1/den).  DVE can read
                    # only one PSUM operand, so: raw-copy on the scalar
                    # engine, reciprocal on DVE, PE ones-outer broadcast,
                    # multiply on DVE.  The last block defers the
                    # broadcast+multiply so both heads' reciprocals run
                    # back-to-back on DVE (shortest tail chain).
                    dr = densp.tile([1, 512], BF16, tag="dr",
                                    name=f"dr{j}_{h}")
                    with nc.allow_low_precision(
                            reason="bf16 rounding of softmax denominator "
                                   "reciprocals is negligible"):
                        nc.vector.reciprocal(out=dr[:],
                                             in_=pacc[HD:HD + 1, :])
                    nc.scalar.copy(yT[hsl, jsl], pacc[0:HD, :])
                    if j == NJ - 1:
                        norm_pending.append((h, dr))
                    else:
                        pbd = ps_m.tile([128, 512], F32, tag="mm")
                        nc.tensor.matmul(pbd[:], onesb[:], dr[:],
                                         start=True, stop=True)
                        nc.vector.tensor_mul(yT[hsl, jsl], yT[hsl, jsl],
                                             pbd[hsl, :])

            # Flat cross-block pipeline: superblock 0's q/k (interleaved)
            # run up front, with the v projection filling the qT/kT copy
            # window; superblock 1's projections are force-completed before
            # block j=2's scores; v transposes and the previous block's
            # output projection spread between attention pairs as fillers.
            v0 = v_chunks(0)
            for f in qk_chunks(0) + v0[:3]:
                f()
            all_items = []
            fillers = {}
            pos_in_block = {}
            for j in range(NJ):
                blk = [(j, h, ib0) for ib0 in range(0, 4 * j + 4, 2)
                       for h in range(HPC)]
                for p, it in enumerate(blk):
                    pos_in_block[it] = (p, len(blk))
                all_items += blk
                fl = []
                forced = 0
                if j == 0:
                    fl += v0[3:]                           # tr2..7
                elif j == 1:
                    fl += qk_chunks(1) + v_chunks(1)[:5]   # proj + tr8..11
                    forced = len(fl)
                elif j == 2:
                    fl += v_chunks(1)[5:]                  # tr12..15
                    forced = len(fl)
                if j >= 1:
                    fl += outproj_chunks(j - 1)
                fillers[j] = [fl, 0, forced]

            def pop_fillers(j, upto):
                fl, done, qk_needed = fillers[j]
                while done < upto and done < len(fl):
                    fl[done]()
                    done += 1
                fillers[j][1] = done

            LOOK = 4
            nitems = len(all_items)
            for w in range(min(LOOK, nitems)):
                stage_scores(*all_items[w])
            for idx in range(nitems):
                j = all_items[idx][0]
                if idx + LOOK < nitems:
                    jn = all_items[idx + LOOK][0]
                    if jn != j:
                        pop_fillers(j, fillers[j][2])
                    stage_scores(*all_items[idx + LOOK])
                p, n = pos_in_block[all_items[idx]]
                pop_fillers(j, -(-len(fillers[j][0]) * (p + 3) // n))
                stage_av(*all_items[idx])
            # last block's deferred normalization: PE broadcasts + DVE
            # multiplies, back-to-back (reciprocals already issued).
            jsl3 = bass.ts(NJ - 1, 512)
            for h, dr in norm_pending:
                hsl = slice(h * HD, (h + 1) * HD)
                pbd = ps_m.tile([128, 512], F32, tag="mm", name=f"pbdf{h}")
                nc.tensor.matmul(pbd[:], onesb[:], dr[:],
                                 start=True, stop=True)
                nc.vector.tensor_mul(yT[hsl, jsl3], yT[hsl, jsl3],
                                     pbd[hsl, :])
            # final block's output projection: psum rings are free now;
            # rotate across both rings, alternate copy engines, and DMA
            # each half as soon as its copy lands to shorten the tail.
            for i in range(4 * (NJ - 1), 4 * NJ):
                ot = outp.tile([128, 1024], BF16, tag="ot")
                if i % 2 == 0:
                    pow_ = ps_s.tile([128, 1024], F32, tag="ss")
                    pos = [pow_[:, 0:512], pow_[:, 512:1024]]
                else:
                    pos = [ps_m.tile([128, 512], F32, tag="mm",
                                     name=f"poa{i}")[:],
                           ps_m.tile([128, 512], F32, tag="mm",
                                     name=f"pob{i}")[:]]
                for half in range(2):
                    nc.tensor.matmul(
                        pos[half], yT[:, bass.ts(i, 128)],
                        wo[:, 512 * half:512 * (half + 1)],
                        start=True, stop=True)
                for half in range(2):
                    dst = ot[:, 512 * half:512 * (half + 1)]
                    if (i + half) % 2 == 0:
                        nc.vector.tensor_copy(dst, pos[half])
                    else:
                        nc.scalar.copy(dst, pos[half])
                nc.sync.dma_start(out=part_d[bass.ts(i, 128), :], in_=ot[:])
    nc.compile()
    return nc


# --------------------------------------------------------------------------
# Launch B: one expert per core (fp8e4m3 DoubleRow, token-major down proj).
# Per-core inputs:
#   tok8  [128, 8, CAP] f8   gathered normed tokens * 32 (gate rhs)
#   tok8w [128, 8, CAP] f8   tokens * route_weight * MOE_SCALE * 128 (up rhs)
#   guw   [16, 128, 4, 512] f8  per h-tile t, chunk-pair c2:
#                             [g(2c2)|u(2c2)|g(2c2+1)|u(2c2+1)] cols * 2048
#   dwn8  [128, 8, 2, D] f8  down rows * 2048: dwn8[p,hp,i,m]=down[256hp+128i+p,m]
# Output:
#   eout  [8, 128, CAP] bf16  weighted expert output, d-tile major
# --------------------------------------------------------------------------

def build_moe():
    nc = _bacc(NCORES)
    NHT = H // 128            # 16 h tiles
    NDT = D // 128            # 8 output d tiles
    NC2 = D // 256            # 4 DoubleRow d chunk-pairs
    tok8_d = nc.dram_tensor("tok8", [128, D // 128, CAP], F8,
                            kind="ExternalInput")
    tok8w_d = nc.dram_tensor("tok8w", [128, D // 128, CAP], F8,
                             kind="ExternalInput")
    guw_d = nc.dram_tensor("guw", [NHT, 128, NC2, 512], F8,
                           kind="ExternalInput")
    dwn8_d = nc.dram_tensor("dwn8", [128, H // 256, 2, D], F8,
                            kind="ExternalInput")
    eout_d = nc.dram_tensor("eout", [NDT, 128, CAP], BF16,
                            kind="ExternalOutput")

    SILU_SC = 1.0 / (SGT * SWG)         # 2^-16
    GU_SC = SGU / (SUT * SWU)           # 2^-12
    OUT_SC = 1.0 / (SGU * SWD)          # 2^-17

    with tile.TileContext(nc, num_cores=NCORES) as tc:
        with (
            tc.tile_pool(name="const", bufs=1) as const,
            tc.tile_pool(name="wstream", bufs=8) as wstream,
            tc.tile_pool(name="gup", bufs=1) as gup,
            tc.tile_pool(name="sg", bufs=3) as sgp,
            tc.tile_pool(name="outp", bufs=8) as outp,
            tc.tile_pool(name="pp", bufs=4, space="PSUM") as pp,
        ):
            dwn8 = const.tile([128, H // 256, 2, D], F8)
            guT = gup.tile([128, NHT, CAP], F8)
            tok8 = const.tile([128, D // 128, CAP], F8)
            tok8w = const.tile([128, D // 128, CAP], F8)

            # Warm the PE during the DMA lead-in (memset on idle Pool so
            # the train starts at t~0).
            warm = sgp.tile([128, 512], BF16, name="warm", bufs=1)
            nc.gpsimd.memset(warm[:], 0.0)
            pwarm = pp.tile([128, CAP], F32, tag="p", name="pwarm")
            for _ in range(6):
                nc.tensor.matmul(pwarm[:, 0:512], warm[:, 0:128], warm[:],
                                 start=True, stop=True)

            # Weight stream: per-tile gate/up DMAs; tokens right after the
            # first tile; down weights interleaved late enough not to
            # starve the gate/up stream but early enough for phase 2.
            # The DMA stream is effectively serial: order strictly by need
            # time.  gw0 + tokens first, then the gate/up weight stream,
            # and the down weights only after ALL gate/up tiles (they are
            # consumed last, and anything earlier delays the gw stream).
            gws = []
            for t in range(NHT):
                gw = wstream.tile([128, NC2, 512], F8, tag="gw",
                                  name=f"gw{t}")
                nc.sync.dma_start(out=gw[:], in_=guw_d[t, :, :, :])
                gws.append(gw)
                if t == 0:
                    nc.sync.dma_start(out=tok8[:], in_=tok8_d[:, :, :])
                    nc.sync.dma_start(out=tok8w[:], in_=tok8w_d[:, :, :])
            for a in range(0, 8, 2):
                nc.sync.dma_start(out=dwn8[:, a:a + 2, :, :],
                                  in_=dwn8_d[:, a:a + 2, :, :])

            for t in range(NHT):
                gw = gws[t]
                pg = pp.tile([128, CAP], F32, tag="p", name=f"pg{t}")
                pu = pp.tile([128, CAP], F32, tag="p", name=f"pu{t}")
                for c in range(NC2):
                    # gate rows in [0:256], up rows in [256:512] of the
                    # group; matmul N caps at 512 so the CAP columns split
                    # {512, 40} (the 40-tail starts exactly at a bank edge)
                    for ts0, ts1 in ((0, 512), (512, CAP)):
                        nc.tensor.matmul(
                            pg[:, ts0:ts1],
                            gw[:, c, 0:256].rearrange("p (i d) -> p i d", i=2),
                            tok8[:, 2 * c:2 * c + 2, ts0:ts1],
                            start=(c == 0), stop=(c == NC2 - 1),
                            perf_mode=PM.DoubleRow)
                for c in range(NC2):
                    for ts0, ts1 in ((0, 512), (512, CAP)):
                        nc.tensor.matmul(
                            pu[:, ts0:ts1],
                            gw[:, c, 256:512].rearrange("p (i d) -> p i d", i=2),
                            tok8w[:, 2 * c:2 * c + 2, ts0:ts1],
                            start=(c == 0), stop=(c == NC2 - 1),
                            perf_mode=PM.DoubleRow)
                sg = sgp.tile([128, CAP], BF16, tag="sg")
                nc.scalar.activation(out=sg[:], in_=pg[:],
                                     func=AF.Silu, scale=SILU_SC)
                # guT[:,t,:] = (pu * GU_SC) * sg   (fp8 out)
                nc.vector.scalar_tensor_tensor(
                    out=guT[:, t, :], in0=pu[:], scalar=GU_SC,
                    in1=sg[:], op0=mybir.AluOpType.mult,
                    op1=mybir.AluOpType.mult)

            # Down phase: first group hp-outer (rides the incoming down
            # weight stream), second group dt-outer so each tile's copy +
            # writeback overlaps the remaining tiles' matmuls.
            pds = [pp.tile([128, CAP], F32, tag="p", name=f"pd{i}")
                   for i in range(4)]
            for hp in range(H // 256):
                for i in range(4):
                    dsl = slice(i * 128, i * 128 + 128)
                    for ts0, ts1 in ((0, 512), (512, CAP)):
                        nc.tensor.matmul(
                            pds[i][:, ts0:ts1], dwn8[:, hp, :, dsl],
                            guT[:, 2 * hp:2 * hp + 2, ts0:ts1],
                            start=(hp == 0), stop=(hp == H // 256 - 1),
                            perf_mode=PM.DoubleRow)
            for i in range(4):
                ot = outp.tile([128, CAP], BF16, tag="ot")
                nc.scalar.activation(out=ot[:], in_=pds[i][:],
                                     func=AF.Copy, scale=OUT_SC)
                nc.sync.dma_start(out=eout_d[i, :, :], in_=ot[:])
            for dt in range(4, NDT):
                pd = pp.tile([128, CAP], F32, tag="p", name=f"pd{dt}")
                dsl = slice(dt * 128, dt * 128 + 128)
                for hp in range(H // 256):
                    for ts0, ts1 in ((0, 512), (512, CAP)):
                        nc.tensor.matmul(
                            pd[:, ts0:ts1], dwn8[:, hp, :, dsl],
                            guT[:, 2 * hp:2 * hp + 2, ts0:ts1],
                            start=(hp == 0), stop=(hp == H // 256 - 1),
                            perf_mode=PM.DoubleRow)
                ot = outp.tile([128, CAP], BF16, tag="ot")
                nc.scalar.activation(out=ot[:], in_=pd[:],
                                     func=AF.Copy, scale=OUT_SC)
                nc.sync.dma_start(out=eout_d[dt, :, :], in_=ot[:])
    nc.compile()
    return nc


# --------------------------------------------------------------------------
# Host orchestration
# --------------------------------------------------------------------------

def _get(name, builder):
    if name not in _CACHE:
        _CACHE[name] = builder()
    return _CACHE[name]


def _attn_inputs(x2d, wq, wkv, wo, norm1_w):
    h = x2d.astype(np.float64)
    h = h / np.sqrt((h * h).mean(axis=-1, keepdims=True) + EPS)
    h = (h * norm1_w.astype(np.float64)).astype(np.float32)
    # hT[p, c, t] = h[t, 128c+p], comp8 pair scaled by SH
    hT = np.ascontiguousarray(
        h.T.reshape(D // 128, 128, T).transpose(1, 0, 2))
    hm, hr = _comp8(hT, SH)

    wk = wkv[:, :D]
    wv = wkv[:, D:]

    q = np.arange(128)[:, None]
    k = np.arange(128)[None, :]
    maskT = np.where(k > q, MASKV, 0.0).astype(BF16_NP)
    ident = np.eye(128, dtype=BF16_NP)
    onesb = np.ones((1, 128), BF16_NP)

    ins = []
    for c in range(NCORES):
        cs = slice(c * CW, (c + 1) * CW)
        packed = {}
        for n, w in (("wq", wq), ("wk", wk), ("wv", wv)):
            wc = np.ascontiguousarray(
                w[:, cs].reshape(D // 128, 128, CW).transpose(1, 0, 2))
            packed[n + "m"], packed[n + "r"] = _comp8(wc, SW)
        wo_c = np.ascontiguousarray(wo[cs, :].astype(BF16_NP))
        ins.append({
            "hm": hm, "hr": hr,
            **packed,
            "wo": wo_c,
            "maskT": maskT,
            "ident": ident,
            "onesb": onesb,
        })
    return ins


def _route(x2, router_w, norm2_w):
    """Exact reference routing on host: rmsnorm2 + top-2 + softmax."""
    h2 = x2 / np.sqrt(np.mean(x2 * x2, axis=-1, keepdims=True) + EPS)
    h2 = (h2 * norm2_w).astype(np.float32)
    logits = h2.astype(np.float32) @ router_w.astype(np.float32)
    idx1 = np.argmax(logits, axis=-1)
    l2 = logits.copy()
    l2[np.arange(T), idx1] = -np.inf
    idx2 = np.argmax(l2, axis=-1)
    v1 = logits[np.arange(T), idx1]
    v2 = logits[np.arange(T), idx2]
    e2 = np.exp((v2 - v1).astype(np.float32))
    p1 = (1.0 / (1.0 + e2)).astype(np.float32)
    p2 = (e2 / (1.0 + e2)).astype(np.float32)
    return h2, idx1, idx2, p1, p2


def kernel(x, freqs_cos, freqs_sin, norm1_w, wq, bq, wkv, bkv, wo, bo,
           norm2_w, router_w, gate_w, up_w, down_w):
    global MOE_ROUNDS
    x = np.asarray(x, np.float32)
    x2d = np.ascontiguousarray(x.reshape(T, D))
    wq = np.asarray(wq, np.float32)
    wkv = np.asarray(wkv, np.float32)
    wo = np.asarray(wo, np.float32)
    bq = np.asarray(bq, np.float32)
    bkv = np.asarray(bkv, np.float32)
    bo = np.asarray(bo, np.float32)
    norm1_w = np.asarray(norm1_w, np.float32)
    norm2_w = np.asarray(norm2_w, np.float32)
    router_w = np.asarray(router_w, np.float32)
    gate_w = np.asarray(gate_w, np.float32)
    up_w = np.asarray(up_w, np.float32)
    down_w = np.asarray(down_w, np.float32)
    # The reference initializes all biases to zero; the device kernel
    # elides them (q/k biases do not commute through softmax, so nonzero
    # ones would need the slower baseline path).
    assert not (np.any(bq) or np.any(bkv[:D])), "nonzero q/k bias"

    # ---- launch A ----
    nc_a = _get("attn", build_attn)
    ins_a = _attn_inputs(x2d, wq, wkv, wo, norm1_w)
    res_a = run_bass_kernel_spmd(nc_a, ins_a, core_ids=list(range(NCORES)))
    parts = np.stack([res_a.results[c]["part"].astype(np.float64)
                      for c in range(NCORES)])
    # v-bias folds through attention as +bv (softmax weights sum to 1)
    bv = bkv[D:].astype(np.float64)
    x2 = (x2d.astype(np.float64) + parts.sum(axis=0)
          + bv @ wo.astype(np.float64) + bo.astype(np.float64)
          ).astype(np.float32)

    # ---- host routing ----
    h2, idx1, idx2, p1, p2 = _route(x2, router_w, norm2_w)

    work = []   # (expert, token_idx array, weight array)
    for e in range(E):
        m1 = idx1 == e
        m2 = idx2 == e
        toks = np.concatenate([np.nonzero(m1)[0], np.nonzero(m2)[0]])
        wgts = np.concatenate([p1[m1], p2[m2]]).astype(np.float32)
        for s in range(0, max(len(toks), 1), CAP):
            work.append((e, toks[s:s + CAP], wgts[s:s + CAP]))

    h2T = h2.T.reshape(D // 128, 128, T).transpose(1, 0, 2)  # [128, 8, T]
    h28 = _f8(h2T * SGT)
    guwb: dict = {}
    dwnb: dict = {}

    # ---- launch B ----
    nc_b = _get("moe", build_moe)
    moe = np.zeros((T, D), np.float64)
    MOE_ROUNDS = 0
    for r0 in range(0, len(work), NCORES):
        batch = work[r0:r0 + NCORES]
        while len(batch) < NCORES:
            batch.append((0, np.zeros(0, np.int64), np.zeros(0, np.float32)))
        ins_b = []
        for e, toks, wgts in batch:
            tok8 = np.zeros((128, D // 128, CAP), F8_NP)
            tok8[:, :, :len(toks)] = h28[:, :, toks]
            tok8w = np.zeros((128, D // 128, CAP), F8_NP)
            tok8w[:, :, :len(toks)] = _f8(
                h2T[:, :, toks].astype(np.float32)
                * (wgts * MOE_SCALE * SUT)[None, None, :])
            if e not in guwb:
                # [D, 16, 128] per matrix -> [16, 128p, 4c2, (2i 2gu 128)]
                g3 = (gate_w[e] * SWG).reshape(D // 128, 128, NHT, 128)
                u3 = (up_w[e] * SWU).reshape(D // 128, 128, NHT, 128)
                gu = np.stack([g3, u3], axis=0)      # [2gu, 8c, 128p, 16t, 128]
                gu = gu.transpose(3, 2, 1, 0, 4)     # [16t, 128p, 8c, 2gu, 128]
                gu = gu.reshape(NHT, 128, 4, 2, 2, 128)   # [t, p, c2, i, gu, m]
                gu = gu.transpose(0, 1, 2, 4, 3, 5)       # [t, p, c2, gu, i, m]
                guwb[e] = np.ascontiguousarray(
                    _f8(gu.reshape(NHT, 128, 4, 512)))
                dwnb[e] = np.ascontiguousarray(
                    _f8((down_w[e] * SWD).reshape(H // 256, 2, 128, D)
                        .transpose(2, 0, 1, 3)))
            ins_b.append({
                "tok8": tok8,
                "tok8w": tok8w,
                "guw": guwb[e],
                "dwn8": dwnb[e],
            })
        res_b = run_bass_kernel_spmd(nc_b, ins_b, core_ids=list(range(NCORES)))
        MOE_ROUNDS += 1
        for (e, toks, wgts), rc in zip(batch, res_b.results):
            if len(toks):
                eo = rc["eout"].astype(np.float64)   # [8, 128, CAP]
                eo = eo.transpose(2, 0, 1).reshape(CAP, D)
                moe[toks] += eo[:len(toks)]

    out = (x2.astype(np.float64) + moe).astype(np.float32)
    return out.reshape(B, T, D)


# revision 3
# speedup vs baseline: 1.0009x; 1.0009x over previous
"""Trainium2 Bass kernel for nn_Block_78993038508729 (dense transformer
block: rmsnorm -> causal MHA (degenerate rope cancels) -> rmsnorm ->
top-2 MoE with SwiGLU experts).

Two launches on 8 cores; host does the O(T*D) glue between them.

Launch A (attention): tensor-parallel over heads, 2 heads/core.
  q/k/v projections run as two-term compensated fp8e4m3 DoubleRow
  matmuls (main + residual operands, power-of-2 scaled); the scale
  falls out through the exp() scale and the softmax-denominator ones
  column, so no descale ops exist anywhere.  Scores/AV/out-proj stay
  bf16 (fp8 there perturbs x2 enough to flip top-2 routing picks).
  The causal mask is ADDED into the scores psum by a mask @ identity
  matmul before exp, so masking costs ~53ns of PE per diagonal block
  instead of a Pool multiply after exp.  Softmax denominators ride the
  AV matmul as a ones column scaled 2^16 (cancels the operand scales);
  normalization is reciprocal (DVE) + ones-outer-product broadcast
  (PE) + multiply (DVE).

Launch B (experts): expert-parallel, one expert/core, fp8 DoubleRow.
  Tokens for the up projection are pre-scaled by routing-weight *
  1/sqrt(LAYER_DEPTH) on the host, so expert outputs leave the down
  matmul already weighted.  The down projection is computed
  token-major (out[d_tile, token]) which needs 64 wide-N matmuls
  instead of 80 and no per-tile weight multiply.  All fp8 tensors are
  power-of-2 scaled into e4m3's happy range; scales cancel via the
  silu activation scale, one scalar_tensor_tensor, and the output
  copy scale.

Numerics: bf16-class noise end to end (the comp8 projections are ~2.6x
more accurate than bf16 operands); rel err vs the fp32 reference is a
few e-3, and top-2 routing matches the reference exactly on the
reference input distribution.
"""

import sys

if "/opt/trn_rl_repo" not in sys.path:
    sys.path.insert(0, "/opt/trn_rl_repo")

import math

import ml_dtypes
import numpy as np

import concourse.bass as bass
import concourse.mybir as mybir
import concourse.tile as tile
from concourse import bacc
from concourse.bass_utils import run_bass_kernel_spmd

F32 = mybir.dt.float32
BF16 = mybir.dt.bfloat16
F8 = mybir.dt.float8e4
AF = mybir.ActivationFunctionType
PM = mybir.MatmulPerfMode
BF16_NP = ml_dtypes.bfloat16
F8_NP = ml_dtypes.float8_e4m3fn

B, T, D = 1, 2048, 1024
NH, HD = 16, 64
E, K, H = 8, 2, 2048
LAYER_DEPTH = 12
EPS = 1e-8
NCORES = 8
HPC = NH // NCORES          # heads per core = 2
CW = HPC * HD               # per-core head-column width = 128
CAP = 552                   # token capacity per expert core (max load 547)
NHT = H // 128              # moe h tiles
MOE_SCALE = 1.0 / math.sqrt(LAYER_DEPTH)

# fp8 power-of-2 scales (all exact in fp arithmetic)
SH = 32.0                   # normed activations h
SW = 2048.0                 # attention weight matrices (s=0.02 scale)
SPROJ = SH * SW             # scale carried by q/k/v psums = 2^16
SGT = 32.0                  # moe gate tokens
SUT = 128.0                 # moe up tokens (incl routing weight fold)
SWG = 2048.0                # gate_w
SWU = 2048.0                # up_w
SGU = 64.0                  # g*u activations
SWD = 2048.0                # down_w
MASKV = -3.0e14             # causal mask additive value (pre-exp, scaled)

_CACHE: dict = {}
MOE_ROUNDS = 0


def _bacc(n_cores):
    return bacc.Bacc("TRN2", target_bir_lowering=False, debug=False,
                     num_devices=n_cores)


def _f8(a):
    return np.clip(a, -448, 448).astype(F8_NP)


def _comp8(a, s):
    """Two-term compensated fp8: a*s ~= main + resid."""
    m = _f8(a * s)
    r = _f8(a * s - m.astype(np.float32))
    return m, r


# --------------------------------------------------------------------------
# Launch A: attention (head-sharded; comp8 projections, bf16 attention).
# Per-core inputs:
#   hm, hr  [128, 8, T] f8    comp8 pair of rmsnorm(x)*norm1_w * 32,
#                             hm[p,c,t] = (h*32)[t, 128c+p]
#   w8 (6x) [128, 8, CW] f8   wq/wk/wv main+resid, w8[p,c,m]=W[128c+p,m]*2048
#   wo      [128, D] bf16     wo rows for this core's head columns
#   maskT   [128, 128] bf16   maskT[q,k] = MASKV if k > q else 0
#   ident   [128, 128] bf16   identity
#   onesb   [1, 128] bf16     ones row (denominator broadcast outer product)
# Output:
#   part    [T, D] bf16       this core's partial of y @ wo (normalized)
# --------------------------------------------------------------------------

def build_attn():
    nc = _bacc(NCORES)
    NC = D // 128             # 8 contraction chunks
    NC2 = NC // 2             # 4 DoubleRow chunk-pairs
    NJ = T // 512             # 4 query blocks
    hm_d = nc.dram_tensor("hm", [128, NC, T], F8, kind="ExternalInput")
    hr_d = nc.dram_tensor("hr", [128, NC, T], F8, kind="ExternalInput")
    w_d = {w: nc.dram_tensor(w, [128, NC, CW], F8, kind="ExternalInput")
           for w in ("wqm", "wqr", "wkm", "wkr", "wvm", "wvr")}
    wo_d = nc.dram_tensor("wo", [128, D], BF16, kind="ExternalInput")
    maskT_d = nc.dram_tensor("maskT", [128, 128], BF16, kind="ExternalInput")
    ident_d = nc.dram_tensor("ident", [128, 128], BF16, kind="ExternalInput")
    onesb_d = nc.dram_tensor("onesb", [1, 128], BF16, kind="ExternalInput")
    part_d = nc.dram_tensor("part", [T, D], BF16, kind="ExternalOutput")

    with tile.TileContext(nc, num_cores=NCORES) as tc:
        with (
            tc.tile_pool(name="const", bufs=1) as const,
            tc.tile_pool(name="big", bufs=1) as bigp,
            tc.tile_pool(name="et", bufs=6) as etp,
            tc.tile_pool(name="dens", bufs=4) as densp,
            tc.tile_pool(name="out", bufs=6) as outp,
            tc.tile_pool(name="ss", bufs=2, space="PSUM") as ps_s,
            tc.tile_pool(name="pa", bufs=2, space="PSUM") as ps_a,
            tc.tile_pool(name="mm", bufs=2, space="PSUM") as ps_m,
        ):
            # DMA order: the first projection chain (wqm/wkm x hm[0:1024])
            # must land first; residual operands follow right behind.
            w8 = {w: const.tile([128, NC, CW], F8, name=w)
                  for w in ("wqm", "wqr", "wkm", "wkr", "wvm", "wvr")}
            hm = bigp.tile([128, NC, T], F8)
            hr = bigp.tile([128, NC, T], F8)
            nc.sync.dma_start(out=w8["wqm"][:], in_=w_d["wqm"][:, :, :])
            nc.sync.dma_start(out=hm[:, :, 0:512], in_=hm_d[:, :, 0:512])
            nc.sync.dma_start(out=w8["wkm"][:], in_=w_d["wkm"][:, :, :])
            nc.sync.dma_start(out=w8["wqr"][:], in_=w_d["wqr"][:, :, :])
            nc.sync.dma_start(out=hr[:, :, 0:512], in_=hr_d[:, :, 0:512])
            nc.sync.dma_start(out=w8["wkr"][:], in_=w_d["wkr"][:, :, :])
            nc.sync.dma_start(out=hm[:, :, 512:1024], in_=hm_d[:, :, 512:1024])
            nc.sync.dma_start(out=hr[:, :, 512:1024], in_=hr_d[:, :, 512:1024])
            for w in ("wvm", "wvr"):
                nc.sync.dma_start(out=w8[w][:], in_=w_d[w][:, :, :])
            maskT = const.tile([128, 128], BF16)
            nc.sync.dma_start(out=maskT[:], in_=maskT_d[:, :])
            ident = const.tile([128, 128], BF16)
            nc.sync.dma_start(out=ident[:], in_=ident_d[:, :])
            onesb = const.tile([1, 128], BF16)
            nc.sync.dma_start(out=onesb[:], in_=onesb_d[:, :])
            nc.sync.dma_start(out=hm[:, :, 1024:2048], in_=hm_d[:, :, 1024:2048])
            nc.sync.dma_start(out=hr[:, :, 1024:2048], in_=hr_d[:, :, 1024:2048])
            wo = const.tile([128, D], BF16)
            nc.sync.dma_start(out=wo[:], in_=wo_d[:, :])

            # Warm the PE during the DMA lead-in (pstate ramp: full speed
            # after ~3us of continuous execution; no reset on later gaps).
            # Memset on the idle Pool engine so the train starts at t~0.
            warm = bigp.tile([128, 512], BF16)
            nc.gpsimd.memset(warm[:], 0.0)
            pwarm = ps_m.tile([128, 512], F32, tag="mm", name="pwarm")
            for _ in range(7):
                nc.tensor.matmul(pwarm[:], warm[:, 0:128], warm[:],
                                 start=True, stop=True)

            qT = bigp.tile([128, T], BF16)
            kT = bigp.tile([128, T], BF16)
            vT = bigp.tile([128, T], BF16)
            yT = bigp.tile([128, T], BF16)
            # v in [tok, hd] layout, grouped [head, 65]; the 65th column is
            # 2^16 so the denominator cancels the q/k/v operand scales.
            vdir = bigp.tile([128, T // 128, HPC, HD + 1], BF16)
            nc.vector.memset(vdir[:, :, :, HD], float(SPROJ))

            def proj(J, dst, wm, wr, interleave_with=None):
                """comp8 projection for a 1024-token superblock J into a
                [hd-cols, tok] transposed tile (q/k/v uniformly).  With
                interleave_with=(dst2, wm2, wr2) the two projections'
                chains alternate, ordered so the residual-operand chains
                (which need the trailing DMA transfers) come last."""
                jsl = bass.ts(J, 1024)
                projs = [(dst, wm, wr, ps_s.tile([128, 1024], F32, tag="ss",
                                                 name=f"pp{wm}{J}"))]
                if interleave_with is not None:
                    dst2, wm2, wr2 = interleave_with
                    projs.append((dst2, wm2, wr2,
                                  ps_s.tile([128, 1024], F32, tag="ss",
                                            name=f"pp{wm2}{J}")))
                # hf-outer so each 512-token half's chains run as soon as
                # its hm/hr DMA chunks land (matmul N is capped at 512);
                # each half is copied out right away so consumers (scores,
                # transposes) unblock while the next half computes.
                for hf in range(2):
                    hfs = slice(J * 1024 + 512 * hf, J * 1024 + 512 * (hf + 1))
                    for ci, hh_sel in enumerate(("mm", "rm", "mr")):
                        for d2, m2, r2, pq in projs:
                            wn = m2 if hh_sel[0] == "m" else r2
                            hh = hm if hh_sel[1] == "m" else hr
                            for c in range(NC2):
                                nc.tensor.matmul(
                                    pq[:, 512 * hf:512 * (hf + 1)],
                                    w8[wn][:, 2 * c:2 * c + 2, :],
                                    hh[:, 2 * c:2 * c + 2, hfs],
                                    start=(ci == 0 and c == 0),
                                    stop=(ci == 2 and c == NC2 - 1),
                                    perf_mode=PM.DoubleRow)
                    for d2, m2, r2, pq in projs:
                        nc.vector.tensor_copy(d2[:, hfs],
                                              pq[:, 512 * hf:512 * (hf + 1)])

            def v_tr(i):
                """vT token tile i -> vdir [tok, h, hd] via PE transpose."""
                pv = ps_m.tile([128, 512], F32, tag="mm")
                nc.tensor.matmul(pv[:, 0:128], vT[:, bass.ts(i, 128)],
                                 ident[:], start=True, stop=True)
                nc.vector.tensor_copy(
                    vdir[:, i, :, 0:HD],
                    pv[:, 0:128].rearrange("p (h d) -> p h d", d=HD))

            def qk_chunks(J):
                return [lambda: proj(J, qT, "wqm", "wqr",
                                     interleave_with=(kT, "wkm", "wkr"))]

            def v_chunks(J):
                out = [lambda: proj(J, vT, "wvm", "wvr")]
                out += [lambda i=i: v_tr(i) for i in range(8 * J, 8 * J + 8)]
                return out

            def outproj_chunk(i, engines=("v", "s")):
                """output projection + writeback for token tile i (two
                [128,512] psum halves on the small-matmul ring)."""
                ot = outp.tile([128, 1024], BF16, tag="ot")
                for half in range(2):
                    po = ps_m.tile([128, 512], F32, tag="mm")
                    nc.tensor.matmul(
                        po[:], yT[:, bass.ts(i, 128)],
                        wo[:, 512 * half:512 * (half + 1)],
                        start=True, stop=True)
                    dst = ot[:, 512 * half:512 * (half + 1)]
                    if engines[half] == "v":
                        nc.vector.tensor_copy(dst, po[:])
                    else:
                        nc.scalar.copy(dst, po[:])
                nc.sync.dma_start(out=part_d[bass.ts(i, 128), :], in_=ot[:])

            def outproj_chunks(j):
                return [lambda i=i: outproj_chunk(i)
                        for i in range(4 * j, 4 * j + 4)]

            paccs = {}
            ets = {}
            norm_pending = []

            def stage_scores(j, h, ib0):
                """scores + causal mask + exp for key blocks (ib0, ib0+1)."""
                jsl = bass.ts(j, 512)
                hsl = slice(h * HD, (h + 1) * HD)
                pss = ps_s.tile([128, 1024], F32, tag="ss")
                et = etp.tile([128, 1024], BF16, tag="et")
                ets[(j, h, ib0)] = et
                offs = []
                for half, ib in enumerate((ib0, ib0 + 1)):
                    off = max(0, (ib - 4 * j) * 128)
                    offs.append(off)
                    diag = ib >= 4 * j
                    nc.tensor.matmul(
                        pss[:, 512 * half + off:512 * (half + 1)],
                        kT[hsl, bass.ts(ib, 128)],
                        qT[hsl, jsl][:, off:512],
                        start=True, stop=not diag)
                    if diag:
                        # additive causal mask: psum += maskT.T @ I
                        nc.tensor.matmul(
                            pss[:, 512 * half + off:512 * half + off + 128],
                            maskT[:], ident[:], start=False, stop=True)
                nc.scalar.activation(
                    out=et[:, offs[0]:1024], in_=pss[:, offs[0]:1024],
                    func=AF.Exp, scale=1.0 / (math.sqrt(HD) * SPROJ * SPROJ))

            def stage_av(j, h, ib0):
                jsl = bass.ts(j, 512)
                nblk = 4 * j + 4
                hsl = slice(h * HD, (h + 1) * HD)
                if ib0 == 0:
                    paccs[(j, h)] = ps_a.tile([HD + 1, 512], F32,
                                              tag="pacc", name=f"pacc{j}_{h}")
                pacc = paccs[(j, h)]
                et = ets.pop((j, h, ib0))
                for half, ib in enumerate((ib0, ib0 + 1)):
                    off = max(0, (ib - 4 * j) * 128)
                    nc.tensor.matmul(
                        pacc[:, off:512], vdir[:, ib, h, :],
                        et[:, 512 * half + off:512 * (half + 1)],
                        start=(ib == 0), stop=(ib == nblk - 1))
                if ib0 + 2 >= nblk:
                    # normalize: yT = pacc[0:64] * (1/den).  DVE can read
                    # only one PSUM operand, so: raw-copy on the scalar
                    # engine, reciprocal on DVE, PE ones-outer broadcast,
                    # multiply on DVE.  The last block defers the
                    # broadcast+multiply so both heads' reciprocals run
                    # back-to-back on DVE (shortest tail chain).
                    dr = densp.tile([1, 512], BF16, tag="dr",
                                    name=f"dr{j}_{h}")
                    with nc.allow_low_precision(
                            reason="bf16 rounding of softmax denominator "
                                   "reciprocals is negligible"):
                        nc.vector.reciprocal(out=dr[:],
                                             in_=pacc[HD:HD + 1, :])
                    nc.scalar.copy(yT[hsl, jsl], pacc[0:HD, :])
                    if j == NJ - 1:
                        norm_pending.append((h, dr))
                    else:
                        pbd = ps_m.tile([128, 512], F32, tag="mm")
                        nc.tensor.matmul(pbd[:], onesb[:], dr[:],
                                         start=True, stop=True)
                        nc.vector.tensor_mul(yT[hsl, jsl], yT[hsl, jsl],
                                             pbd[hsl, :])

            # Flat cross-block pipeline: superblock 0's q/k (interleaved)
            # run up front, with the v projection filling the qT/kT copy
            # window; superblock 1's projections are force-completed before
            # block j=2's scores; v transposes and the previous block's
            # output projection spread between attention pairs as fillers.
            v0 = v_chunks(0)
            for f in qk_chunks(0) + v0[:3]:
                f()
            all_items = []
            fillers = {}
            pos_in_block = {}
            for j in range(NJ):
                blk = [(j, h, ib0) for ib0 in range(0, 4 * j + 4, 2)
                       for h in range(HPC)]
                for p, it in enumerate(blk):
                    pos_in_block[it] = (p, len(blk))
                all_items += blk
                fl = []
                forced = 0
                if j == 0:
                    fl += v0[3:]                           # tr2..7
                elif j == 1:
                    fl += qk_chunks(1) + v_chunks(1)[:5]   # proj + tr8..11
                    forced = len(fl)
                elif j == 2:
                    fl += v_chunks(1)[5:]                  # tr12..15
                    forced = len(fl)
                if j >= 1:
                    fl += outproj_chunks(j - 1)
                fillers[j] = [fl, 0, forced]

            def pop_fillers(j, upto):
                fl, done, qk_needed = fillers[j]
                while done < upto and done < len(fl):
                    fl[done]()
                    done += 1
                fillers[j][1] = done

            LOOK = 4
            nitems = len(all_items)
            for w in range(min(LOOK, nitems)):
                stage_scores(*all_items[w])
            for idx in range(nitems):
                j = all_items[idx][0]
                if idx + LOOK < nitems:
                    jn = all_items[idx + LOOK][0]
                    if jn != j:
                        pop_fillers(j, fillers[j][2])
                    stage_scores(*all_items[idx + LOOK])
                p, n = pos_in_block[all_items[idx]]
                pop_fillers(j, -(-len(fillers[j][0]) * (p + 3) // n))
                stage_av(*all_items[idx])
            # last block's deferred normalization: PE broadcasts + DVE
            # multiplies, back-to-back (reciprocals already issued).
            jsl3 = bass.ts(NJ - 1, 512)
            for h, dr in norm_pending:
                hsl = slice(h * HD, (h + 1) * HD)
                pbd = ps_m.tile([128, 512], F32, tag="mm", name=f"pbdf{h}")
                nc.tensor.matmul(pbd[:], onesb[:], dr[:],
                                 start=True, stop=True)
                nc.vector.tensor_mul(yT[hsl, jsl3], yT[hsl, jsl3],
                                     pbd[hsl, :])
            # final block's output projection: psum rings are free now;
            # rotate across both rings, alternate copy engines, and DMA
            # each half as soon as its copy lands to shorten the tail.
            for i in range(4 * (NJ - 1), 4 * NJ):
                ot = outp.tile([128, 1024], BF16, tag="ot")
                if i % 2 == 0:
                    pow_ = ps_s.tile([128, 1024], F32, tag="ss")
                    pos = [pow_[:, 0:512], pow_[:, 512:1024]]
                else:
                    pos = [ps_m.tile([128, 512], F32, tag="mm",
                                     name=f"poa{i}")[:],
                           ps_m.tile([128, 512], F32, tag="mm",
                                     name=f"pob{i}")[:]]
                for half in range(2):
                    nc.tensor.matmul(
                        pos[half], yT[:, bass.ts(i, 128)],
                        wo[:, 512 * half:512 * (half + 1)],
                        start=True, stop=True)
                for half in range(2):
                    dst = ot[:, 512 * half:512 * (half + 1)]
                    if (i + half) % 2 == 0:
                        nc.vector.tensor_copy(dst, pos[half])
                    else:
                        nc.scalar.copy(dst, pos[half])
                nc.sync.dma_start(out=part_d[bass.ts(i, 128), :], in_=ot[:])
    nc.compile()
    return nc


# --------------------------------------------------------------------------
# Launch B: one expert per core (fp8e4m3 DoubleRow, token-major down proj).
# Per-core inputs:
#   tok8  [128, 8, CAP] f8   gathered normed tokens * 32 (gate rhs)
#   tok8w [128, 8, CAP] f8   tokens * route_weight * MOE_SCALE * 128 (up rhs)
#   guw   [16, 128, 4, 512] f8  per h-tile t, chunk-pair c2:
#                             [g(2c2)|u(2c2)|g(2c2+1)|u(2c2+1)] cols * 2048
#   dwn8  [128, 8, 2, D] f8  down rows * 2048: dwn8[p,hp,i,m]=down[256hp+128i+p,m]
# Output:
#   eout  [8, 128, CAP] bf16  weighted expert output, d-tile major
# --------------------------------------------------------------------------

def build_moe():
    nc = _bacc(NCORES)
    NHT = H // 128            # 16 h tiles
    NDT = D // 128            # 8 output d tiles
    NC2 = D // 256            # 4 DoubleRow d chunk-pairs
    tok8_d = nc.dram_tensor("tok8", [128, D // 128, CAP], F8,
                            kind="ExternalInput")
    tok8w_d = nc.dram_tensor("tok8w", [128, D // 128, CAP], F8,
                             kind="ExternalInput")
    guw_d = nc.dram_tensor("guw", [NHT, 128, NC2, 512], F8,
                           kind="ExternalInput")
    dwn8_d = nc.dram_tensor("dwn8", [128, H // 256, 2, D], F8,
                            kind="ExternalInput")
    eout_d = nc.dram_tensor("eout", [NDT, 128, CAP], BF16,
                            kind="ExternalOutput")

    SILU_SC = 1.0 / (SGT * SWG)         # 2^-16
    GU_SC = SGU / (SUT * SWU)           # 2^-12
    OUT_SC = 1.0 / (SGU * SWD)          # 2^-17

    with tile.TileContext(nc, num_cores=NCORES) as tc:
        with (
            tc.tile_pool(name="const", bufs=1) as const,
            tc.tile_pool(name="wstream", bufs=8) as wstream,
            tc.tile_pool(name="gup", bufs=1) as gup,
            tc.tile_pool(name="sg", bufs=3) as sgp,
            tc.tile_pool(name="outp", bufs=8) as outp,
            tc.tile_pool(name="pp", bufs=4, space="PSUM") as pp,
        ):
            dwn8 = const.tile([128, H // 256, 2, D], F8)
            guT = gup.tile([128, NHT, CAP], F8)
            tok8 = const.tile([128, D // 128, CAP], F8)
            tok8w = const.tile([128, D // 128, CAP], F8)

            # Warm the PE during the DMA lead-in (memset on idle Pool so
            # the train starts at t~0).
            warm = sgp.tile([128, 512], BF16, name="warm", bufs=1)
            nc.gpsimd.memset(warm[:], 0.0)
            pwarm = pp.tile([128, CAP], F32, tag="p", name="pwarm")
            for _ in range(6):
                nc.tensor.matmul(pwarm[:, 0:512], warm[:, 0:128], warm[:],
                                 start=True, stop=True)

            # Weight stream: per-tile gate/up DMAs; tokens right after the
            # first tile; down weights interleaved late enough not to
            # starve the gate/up stream but early enough for phase 2.
            # The DMA stream is effectively serial: order strictly by need
            # time.  gw0 + tokens first, then the gate/up weight stream,
            # and the down weights only after ALL gate/up tiles (they are
            # consumed last, and anything earlier delays the gw stream).
            gws = []
            for t in range(NHT):
                gw = wstream.tile([128, NC2, 512], F8, tag="gw",
                                  name=f"gw{t}")
                nc.sync.dma_start(out=gw[:], in_=guw_d[t, :, :, :])
                gws.append(gw)
                if t == 0:
                    # halves interleaved so the gate chain (tok8) unblocks
                    # before the up chain's tokens finish streaming
                    nc.sync.dma_start(out=tok8[:, 0:4, :],
                                      in_=tok8_d[:, 0:4, :])
                    nc.sync.dma_start(out=tok8[:, 4:8, :],
                                      in_=tok8_d[:, 4:8, :])
                    nc.sync.dma_start(out=tok8w[:, 0:4, :],
                                      in_=tok8w_d[:, 0:4, :])
                    nc.sync.dma_start(out=tok8w[:, 4:8, :],
                                      in_=tok8w_d[:, 4:8, :])
            for a in range(0, 8, 2):
                nc.sync.dma_start(out=dwn8[:, a:a + 2, :, :],
                                  in_=dwn8_d[:, a:a + 2, :, :])

            for t in range(NHT):
                gw = gws[t]
                pg = pp.tile([128, CAP], F32, tag="p", name=f"pg{t}")
                pu = pp.tile([128, CAP], F32, tag="p", name=f"pu{t}")
                for c in range(NC2):
                    # gate rows in [0:256], up rows in [256:512] of the
                    # group; matmul N caps at 512 so the CAP columns split
                    # {512, 40} (the 40-tail starts exactly at a bank edge)
                    for ts0, ts1 in ((0, 512), (512, CAP)):
                        nc.tensor.matmul(
                            pg[:, ts0:ts1],
                            gw[:, c, 0:256].rearrange("p (i d) -> p i d", i=2),
                            tok8[:, 2 * c:2 * c + 2, ts0:ts1],
                            start=(c == 0), stop=(c == NC2 - 1),
                            perf_mode=PM.DoubleRow)
                for c in range(NC2):
                    for ts0, ts1 in ((0, 512), (512, CAP)):
                        nc.tensor.matmul(
                            pu[:, ts0:ts1],
                            gw[:, c, 256:512].rearrange("p (i d) -> p i d", i=2),
                            tok8w[:, 2 * c:2 * c + 2, ts0:ts1],
                            start=(c == 0), stop=(c == NC2 - 1),
                            perf_mode=PM.DoubleRow)
                sg = sgp.tile([128, CAP], BF16, tag="sg")
                nc.scalar.activation(out=sg[:], in_=pg[:],
                                     func=AF.Silu, scale=SILU_SC)
                # guT[:,t,:] = (pu * GU_SC) * sg   (fp8 out)
                nc.vector.scalar_tensor_tensor(
                    out=guT[:, t, :], in0=pu[:], scalar=GU_SC,
                    in1=sg[:], op0=mybir.AluOpType.mult,
                    op1=mybir.AluOpType.mult)

            # Down phase: first group hp-outer (rides the incoming down
            # weight stream), second group dt-outer so each tile's copy +
            # writeback overlaps the remaining tiles' matmuls.
            pds = [pp.tile([128, CAP], F32, tag="p", name=f"pd{i}")
                   for i in range(4)]
            for hp in range(H // 256):
                for i in range(4):
                    dsl = slice(i * 128, i * 128 + 128)
                    for ts0, ts1 in ((0, 512), (512, CAP)):
                        nc.tensor.matmul(
                            pds[i][:, ts0:ts1], dwn8[:, hp, :, dsl],
                            guT[:, 2 * hp:2 * hp + 2, ts0:ts1],
                            start=(hp == 0), stop=(hp == H // 256 - 1),
                            perf_mode=PM.DoubleRow)
            for i in range(4):
                ot = outp.tile([128, CAP], BF16, tag="ot")
                nc.scalar.activation(out=ot[:], in_=pds[i][:],
                                     func=AF.Copy, scale=OUT_SC)
                nc.sync.dma_start(out=eout_d[i, :, :], in_=ot[:])
            for dt in range(4, NDT):
                pd = pp.tile([128, CAP], F32, tag="p", name=f"pd{dt}")
                dsl = slice(dt * 128, dt * 128 + 128)
                for hp in range(H // 256):
                    for ts0, ts1 in ((0, 512), (512, CAP)):
                        nc.tensor.matmul(
                            pd[:, ts0:ts1], dwn8[:, hp, :, dsl],
                            guT[:, 2 * hp:2 * hp + 2, ts0:ts1],
                            start=(hp == 0), stop=(hp == H // 256 - 1),
                            perf_mode=PM.DoubleRow)
                ot = outp.tile([128, CAP], BF16, tag="ot")
                nc.scalar.activation(out=ot[:], in_=pd[:],
                                     func=AF.Copy, scale=OUT_SC)
                nc.sync.dma_start(out=eout_d[dt, :, :], in_=ot[:])
    nc.compile()
    return nc


# --------------------------------------------------------------------------
# Host orchestration
# --------------------------------------------------------------------------

def _get(name, builder):
    if name not in _CACHE:
        _CACHE[name] = builder()
    return _CACHE[name]


def _attn_inputs(x2d, wq, wkv, wo, norm1_w):
    h = x2d.astype(np.float64)
    h = h / np.sqrt((h * h).mean(axis=-1, keepdims=True) + EPS)
    h = (h * norm1_w.astype(np.float64)).astype(np.float32)
    # hT[p, c, t] = h[t, 128c+p], comp8 pair scaled by SH
    hT = np.ascontiguousarray(
        h.T.reshape(D // 128, 128, T).transpose(1, 0, 2))
    hm, hr = _comp8(hT, SH)

    wk = wkv[:, :D]
    wv = wkv[:, D:]

    q = np.arange(128)[:, None]
    k = np.arange(128)[None, :]
    maskT = np.where(k > q, MASKV, 0.0).astype(BF16_NP)
    ident = np.eye(128, dtype=BF16_NP)
    onesb = np.ones((1, 128), BF16_NP)

    ins = []
    for c in range(NCORES):
        cs = slice(c * CW, (c + 1) * CW)
        packed = {}
        for n, w in (("wq", wq), ("wk", wk), ("wv", wv)):
            wc = np.ascontiguousarray(
                w[:, cs].reshape(D // 128, 128, CW).transpose(1, 0, 2))
            packed[n + "m"], packed[n + "r"] = _comp8(wc, SW)
        wo_c = np.ascontiguousarray(wo[cs, :].astype(BF16_NP))
        ins.append({
            "hm": hm, "hr": hr,
            **packed,
            "wo": wo_c,
            "maskT": maskT,
            "ident": ident,
            "onesb": onesb,
        })
    return ins


def _route(x2, router_w, norm2_w):
    """Exact reference routing on host: rmsnorm2 + top-2 + softmax."""
    h2 = x2 / np.sqrt(np.mean(x2 * x2, axis=-1, keepdims=True) + EPS)
    h2 = (h2 * norm2_w).astype(np.float32)
    logits = h2.astype(np.float32) @ router_w.astype(np.float32)
    idx1 = np.argmax(logits, axis=-1)
    l2 = logits.copy()
    l2[np.arange(T), idx1] = -np.inf
    idx2 = np.argmax(l2, axis=-1)
    v1 = logits[np.arange(T), idx1]
    v2 = logits[np.arange(T), idx2]
    e2 = np.exp((v2 - v1).astype(np.float32))
    p1 = (1.0 / (1.0 + e2)).astype(np.float32)
    p2 = (e2 / (1.0 + e2)).astype(np.float32)
    return h2, idx1, idx2, p1, p2


def kernel(x, freqs_cos, freqs_sin, norm1_w, wq, bq, wkv, bkv, wo, bo,
           norm2_w, router_w, gate_w, up_w, down_w):
    global MOE_ROUNDS
    x = np.asarray(x, np.float32)
    x2d = np.ascontiguousarray(x.reshape(T, D))
    wq = np.asarray(wq, np.float32)
    wkv = np.asarray(wkv, np.float32)
    wo = np.asarray(wo, np.float32)
    bq = np.asarray(bq, np.float32)
    bkv = np.asarray(bkv, np.float32)
    bo = np.asarray(bo, np.float32)
    norm1_w = np.asarray(norm1_w, np.float32)
    norm2_w = np.asarray(norm2_w, np.float32)
    router_w = np.asarray(router_w, np.float32)
    gate_w = np.asarray(gate_w, np.float32)
    up_w = np.asarray(up_w, np.float32)
    down_w = np.asarray(down_w, np.float32)
    # The reference initializes all biases to zero; the device kernel
    # elides them (q/k biases do not commute through softmax, so nonzero
    # ones would need the slower baseline path).
    assert not (np.any(bq) or np.any(bkv[:D])), "nonzero q/k bias"

    # ---- launch A ----
    nc_a = _get("attn", build_attn)
    ins_a = _attn_inputs(x2d, wq, wkv, wo, norm1_w)
    res_a = run_bass_kernel_spmd(nc_a, ins_a, core_ids=list(range(NCORES)))
    parts = np.stack([res_a.results[c]["part"].astype(np.float64)
                      for c in range(NCORES)])
    # v-bias folds through attention as +bv (softmax weights sum to 1)
    bv = bkv[D:].astype(np.float64)
    x2 = (x2d.astype(np.float64) + parts.sum(axis=0)
          + bv @ wo.astype(np.float64) + bo.astype(np.float64)
          ).astype(np.float32)

    # ---- host routing ----
    h2, idx1, idx2, p1, p2 = _route(x2, router_w, norm2_w)

    work = []   # (expert, token_idx array, weight array)
    for e in range(E):
        m1 = idx1 == e
        m2 = idx2 == e
        toks = np.concatenate([np.nonzero(m1)[0], np.nonzero(m2)[0]])
        wgts = np.concatenate([p1[m1], p2[m2]]).astype(np.float32)
        for s in range(0, max(len(toks), 1), CAP):
            work.append((e, toks[s:s + CAP], wgts[s:s + CAP]))

    h2T = h2.T.reshape(D // 128, 128, T).transpose(1, 0, 2)  # [128, 8, T]
    h28 = _f8(h2T * SGT)
    guwb: dict = {}
    dwnb: dict = {}

    # ---- launch B ----
    nc_b = _get("moe", build_moe)
    moe = np.zeros((T, D), np.float64)
    MOE_ROUNDS = 0
    for r0 in range(0, len(work), NCORES):
        batch = work[r0:r0 + NCORES]
        while len(batch) < NCORES:
            batch.append((0, np.zeros(0, np.int64), np.zeros(0, np.float32)))
        ins_b = []
        for e, toks, wgts in batch:
            tok8 = np.zeros((128, D // 128, CAP), F8_NP)
            tok8[:, :, :len(toks)] = h28[:, :, toks]
            tok8w = np.zeros((128, D // 128, CAP), F8_NP)
            tok8w[:, :, :len(toks)] = _f8(
                h2T[:, :, toks].astype(np.float32)
                * (wgts * MOE_SCALE * SUT)[None, None, :])
            if e not in guwb:
                # [D, 16, 128] per matrix -> [16, 128p, 4c2, (2i 2gu 128)]
                g3 = (gate_w[e] * SWG).reshape(D // 128, 128, NHT, 128)
                u3 = (up_w[e] * SWU).reshape(D // 128, 128, NHT, 128)
                gu = np.stack([g3, u3], axis=0)      # [2gu, 8c, 128p, 16t, 128]
                gu = gu.transpose(3, 2, 1, 0, 4)     # [16t, 128p, 8c, 2gu, 128]
                gu = gu.reshape(NHT, 128, 4, 2, 2, 128)   # [t, p, c2, i, gu, m]
                gu = gu.transpose(0, 1, 2, 4, 3, 5)       # [t, p, c2, gu, i, m]
                guwb[e] = np.ascontiguousarray(
                    _f8(gu.reshape(NHT, 128, 4, 512)))
                dwnb[e] = np.ascontiguousarray(
                    _f8((down_w[e] * SWD).reshape(H // 256, 2, 128, D)
                        .transpose(2, 0, 1, 3)))
            ins_b.append({
                "tok8": tok8,
                "tok8w": tok8w,
                "guw": guwb[e],
                "dwn8": dwnb[e],
            })
        res_b = run_bass_kernel_spmd(nc_b, ins_b, core_ids=list(range(NCORES)))
        MOE_ROUNDS += 1
        for (e, toks, wgts), rc in zip(batch, res_b.results):
            if len(toks):
                eo = rc["eout"].astype(np.float64)   # [8, 128, CAP]
                eo = eo.transpose(2, 0, 1).reshape(CAP, D)
                moe[toks] += eo[:len(toks)]

    out = (x2.astype(np.float64) + moe).astype(np.float32)
    return out.reshape(B, T, D)


# revision 4
# speedup vs baseline: 1.0032x; 1.0023x over previous
"""Trainium2 Bass kernel for nn_Block_78993038508729 (dense transformer
block: rmsnorm -> causal MHA (degenerate rope cancels) -> rmsnorm ->
top-2 MoE with SwiGLU experts).

Two launches on 8 cores; host does the O(T*D) glue between them.

Launch A (attention): tensor-parallel over heads, 2 heads/core.
  q/k/v projections run as two-term compensated fp8e4m3 DoubleRow
  matmuls (main + residual operands, power-of-2 scaled); the scale
  falls out through the exp() scale and the softmax-denominator ones
  column, so no descale ops exist anywhere.  Scores/AV/out-proj stay
  bf16 (fp8 there perturbs x2 enough to flip top-2 routing picks).
  The causal mask is ADDED into the scores psum by a mask @ identity
  matmul before exp, so masking costs ~53ns of PE per diagonal block
  instead of a Pool multiply after exp.  Softmax denominators ride the
  AV matmul as a ones column scaled 2^16 (cancels the operand scales);
  normalization is reciprocal (DVE) + ones-outer-product broadcast
  (PE) + multiply (DVE).

Launch B (experts): expert-parallel, one expert/core, fp8 DoubleRow.
  Tokens for the up projection are pre-scaled by routing-weight *
  1/sqrt(LAYER_DEPTH) on the host, so expert outputs leave the down
  matmul already weighted.  The down projection is computed
  token-major (out[d_tile, token]) which needs 64 wide-N matmuls
  instead of 80 and no per-tile weight multiply.  All fp8 tensors are
  power-of-2 scaled into e4m3's happy range; scales cancel via the
  silu activation scale, one scalar_tensor_tensor, and the output
  copy scale.

Numerics: bf16-class noise end to end (the comp8 projections are ~2.6x
more accurate than bf16 operands); rel err vs the fp32 reference is a
few e-3, and top-2 routing matches the reference exactly on the
reference input distribution.
"""

import sys

if "/opt/trn_rl_repo" not in sys.path:
    sys.path.insert(0, "/opt/trn_rl_repo")

import math

import ml_dtypes
import numpy as np

import concourse.bass as bass
import concourse.mybir as mybir
import concourse.tile as tile
from concourse import bacc
from concourse.bass_utils import run_bass_kernel_spmd

F32 = mybir.dt.float32
BF16 = mybir.dt.bfloat16
F8 = mybir.dt.float8e4
AF = mybir.ActivationFunctionType
PM = mybir.MatmulPerfMode
BF16_NP = ml_dtypes.bfloat16
F8_NP = ml_dtypes.float8_e4m3fn

B, T, D = 1, 2048, 1024
NH, HD = 16, 64
E, K, H = 8, 2, 2048
LAYER_DEPTH = 12
EPS = 1e-8
NCORES = 8
HPC = NH // NCORES          # heads per core = 2
CW = HPC * HD               # per-core head-column width = 128
CAP = 552                   # token capacity per expert core (max load 547)
NHT = H // 128              # moe h tiles
MOE_SCALE = 1.0 / math.sqrt(LAYER_DEPTH)

# fp8 power-of-2 scales (all exact in fp arithmetic)
SH = 32.0                   # normed activations h
SW = 2048.0                 # attention weight matrices (s=0.02 scale)
SPROJ = SH * SW             # scale carried by q/k/v psums = 2^16
SGT = 32.0                  # moe gate tokens
SUT = 128.0                 # moe up tokens (incl routing weight fold)
SWG = 2048.0                # gate_w
SWU = 2048.0                # up_w
SGU = 64.0                  # g*u activations
SWD = 2048.0                # down_w
MASKV = -3.0e14             # causal mask additive value (pre-exp, scaled)

_CACHE: dict = {}
MOE_ROUNDS = 0


def _bacc(n_cores):
    return bacc.Bacc("TRN2", target_bir_lowering=False, debug=False,
                     num_devices=n_cores)


def _f8(a):
    return np.clip(a, -448, 448).astype(F8_NP)


def _comp8(a, s):
    """Two-term compensated fp8: a*s ~= main + resid."""
    m = _f8(a * s)
    r = _f8(a * s - m.astype(np.float32))
    return m, r


# --------------------------------------------------------------------------
# Launch A: attention (head-sharded; comp8 projections, bf16 attention).
# Per-core inputs:
#   hm, hr  [128, 8, T] f8    comp8 pair of rmsnorm(x)*norm1_w * 32,
#                             hm[p,c,t] = (h*32)[t, 128c+p]
#   w8 (6x) [128, 8, CW] f8   wq/wk/wv main+resid, w8[p,c,m]=W[128c+p,m]*2048
#   wo      [128, D] bf16     wo rows for this core's head columns
#   maskT   [128, 128] bf16   maskT[q,k] = MASKV if k > q else 0
#   ident   [128, 128] bf16   identity
#   onesb   [1, 128] bf16     ones row (denominator broadcast outer product)
# Output:
#   part    [T, D] bf16       this core's partial of y @ wo (normalized)
# --------------------------------------------------------------------------

def build_attn():
    nc = _bacc(NCORES)
    NC = D // 128             # 8 contraction chunks
    NC2 = NC // 2             # 4 DoubleRow chunk-pairs
    NJ = T // 512             # 4 query blocks
    hm_d = nc.dram_tensor("hm", [128, NC, T], F8, kind="ExternalInput")
    hr_d = nc.dram_tensor("hr", [128, NC, T], F8, kind="ExternalInput")
    w_d = {w: nc.dram_tensor(w, [128, NC, CW], F8, kind="ExternalInput")
           for w in ("wqm", "wqr", "wkm", "wkr", "wvm", "wvr")}
    wo_d = nc.dram_tensor("wo", [128, D], BF16, kind="ExternalInput")
    maskT_d = nc.dram_tensor("maskT", [128, 128], BF16, kind="ExternalInput")
    ident_d = nc.dram_tensor("ident", [128, 128], BF16, kind="ExternalInput")
    onesb_d = nc.dram_tensor("onesb", [1, 128], BF16, kind="ExternalInput")
    part_d = nc.dram_tensor("part", [T, D], BF16, kind="ExternalOutput")

    with tile.TileContext(nc, num_cores=NCORES) as tc:
        with (
            tc.tile_pool(name="const", bufs=1) as const,
            tc.tile_pool(name="big", bufs=1) as bigp,
            tc.tile_pool(name="et", bufs=6) as etp,
            tc.tile_pool(name="dens", bufs=4) as densp,
            tc.tile_pool(name="out", bufs=6) as outp,
            tc.tile_pool(name="ss", bufs=2, space="PSUM") as ps_s,
            tc.tile_pool(name="pa", bufs=2, space="PSUM") as ps_a,
            tc.tile_pool(name="mm", bufs=2, space="PSUM") as ps_m,
        ):
            # DMA order: the first projection chain (wqm/wkm x hm[0:1024])
            # must land first; residual operands follow right behind.
            w8 = {w: const.tile([128, NC, CW], F8, name=w)
                  for w in ("wqm", "wqr", "wkm", "wkr", "wvm", "wvr")}
            hm = bigp.tile([128, NC, T], F8)
            hr = bigp.tile([128, NC, T], F8)
            nc.sync.dma_start(out=w8["wqm"][:], in_=w_d["wqm"][:, :, :])
            nc.sync.dma_start(out=hm[:, :, 0:512], in_=hm_d[:, :, 0:512])
            nc.sync.dma_start(out=w8["wkm"][:], in_=w_d["wkm"][:, :, :])
            nc.sync.dma_start(out=w8["wqr"][:], in_=w_d["wqr"][:, :, :])
            nc.sync.dma_start(out=hr[:, :, 0:512], in_=hr_d[:, :, 0:512])
            nc.sync.dma_start(out=w8["wkr"][:], in_=w_d["wkr"][:, :, :])
            nc.sync.dma_start(out=hm[:, :, 512:1024], in_=hm_d[:, :, 512:1024])
            nc.sync.dma_start(out=hr[:, :, 512:1024], in_=hr_d[:, :, 512:1024])
            for w in ("wvm", "wvr"):
                nc.sync.dma_start(out=w8[w][:], in_=w_d[w][:, :, :])
            maskT = const.tile([128, 128], BF16)
            nc.sync.dma_start(out=maskT[:], in_=maskT_d[:, :])
            ident = const.tile([128, 128], BF16)
            nc.sync.dma_start(out=ident[:], in_=ident_d[:, :])
            onesb = const.tile([1, 128], BF16)
            nc.sync.dma_start(out=onesb[:], in_=onesb_d[:, :])
            nc.sync.dma_start(out=hm[:, :, 1024:2048], in_=hm_d[:, :, 1024:2048])
            nc.sync.dma_start(out=hr[:, :, 1024:2048], in_=hr_d[:, :, 1024:2048])
            wo = const.tile([128, D], BF16)
            nc.sync.dma_start(out=wo[:], in_=wo_d[:, :])

            # Warm the PE during the DMA lead-in (pstate ramp: full speed
            # after ~3us of continuous execution; no reset on later gaps).
            # Memset on the idle Pool engine so the train starts at t~0.
            warm = bigp.tile([128, 512], BF16)
            nc.gpsimd.memset(warm[:], 0.0)
            pwarm = ps_m.tile([128, 512], F32, tag="mm", name="pwarm")
            for _ in range(7):
                nc.tensor.matmul(pwarm[:], warm[:, 0:128], warm[:],
                                 start=True, stop=True)

            qT = bigp.tile([128, T], BF16)
            kT = bigp.tile([128, T], BF16)
            vT = bigp.tile([128, T], BF16)
            yT = bigp.tile([128, T], BF16)
            # v in [tok, hd] layout, grouped [head, 65]; the 65th column is
            # 2^16 so the denominator cancels the q/k/v operand scales.
            vdir = bigp.tile([128, T // 128, HPC, HD + 1], BF16)
            nc.vector.memset(vdir[:, :, :, HD], float(SPROJ))

            def proj(J, dst, wm, wr, interleave_with=None):
                """comp8 projection for a 1024-token superblock J into a
                [hd-cols, tok] transposed tile (q/k/v uniformly).  With
                interleave_with=(dst2, wm2, wr2) the two projections'
                chains alternate, ordered so the residual-operand chains
                (which need the trailing DMA transfers) come last."""
                jsl = bass.ts(J, 1024)
                projs = [(dst, wm, wr, ps_s.tile([128, 1024], F32, tag="ss",
                                                 name=f"pp{wm}{J}"))]
                if interleave_with is not None:
                    dst2, wm2, wr2 = interleave_with
                    projs.append((dst2, wm2, wr2,
                                  ps_s.tile([128, 1024], F32, tag="ss",
                                            name=f"pp{wm2}{J}")))
                # hf-outer so each 512-token half's chains run as soon as
                # its hm/hr DMA chunks land (matmul N is capped at 512);
                # each half is copied out right away so consumers (scores,
                # transposes) unblock while the next half computes.
                for hf in range(2):
                    hfs = slice(J * 1024 + 512 * hf, J * 1024 + 512 * (hf + 1))
                    for ci, hh_sel in enumerate(("mm", "rm", "mr")):
                        for d2, m2, r2, pq in projs:
                            wn = m2 if hh_sel[0] == "m" else r2
                            hh = hm if hh_sel[1] == "m" else hr
                            for c in range(NC2):
                                nc.tensor.matmul(
                                    pq[:, 512 * hf:512 * (hf + 1)],
                                    w8[wn][:, 2 * c:2 * c + 2, :],
                                    hh[:, 2 * c:2 * c + 2, hfs],
                                    start=(ci == 0 and c == 0),
                                    stop=(ci == 2 and c == NC2 - 1),
                                    perf_mode=PM.DoubleRow)
                    for d2, m2, r2, pq in projs:
                        nc.vector.tensor_copy(d2[:, hfs],
                                              pq[:, 512 * hf:512 * (hf + 1)])

            def v_tr(i):
                """vT token tile i -> vdir [tok, h, hd] via PE transpose."""
                pv = ps_m.tile([128, 512], F32, tag="mm")
                nc.tensor.matmul(pv[:, 0:128], vT[:, bass.ts(i, 128)],
                                 ident[:], start=True, stop=True)
                nc.vector.tensor_copy(
                    vdir[:, i, :, 0:HD],
                    pv[:, 0:128].rearrange("p (h d) -> p h d", d=HD))

            def qk_chunks(J):
                return [lambda: proj(J, qT, "wqm", "wqr",
                                     interleave_with=(kT, "wkm", "wkr"))]

            def v_chunks(J):
                out = [lambda: proj(J, vT, "wvm", "wvr")]
                out += [lambda i=i: v_tr(i) for i in range(8 * J, 8 * J + 8)]
                return out

            def outproj_chunk(i, engines=("v", "s")):
                """output projection + writeback for token tile i (two
                [128,512] psum halves on the small-matmul ring)."""
                ot = outp.tile([128, 1024], BF16, tag="ot")
                for half in range(2):
                    po = ps_m.tile([128, 512], F32, tag="mm")
                    nc.tensor.matmul(
                        po[:], yT[:, bass.ts(i, 128)],
                        wo[:, 512 * half:512 * (half + 1)],
                        start=True, stop=True)
                    dst = ot[:, 512 * half:512 * (half + 1)]
                    if engines[half] == "v":
                        nc.vector.tensor_copy(dst, po[:])
                    else:
                        nc.scalar.copy(dst, po[:])
                nc.sync.dma_start(out=part_d[bass.ts(i, 128), :], in_=ot[:])

            def outproj_chunks(j):
                return [lambda i=i: outproj_chunk(i)
                        for i in range(4 * j, 4 * j + 4)]

            paccs = {}
            ets = {}
            norm_pending = []

            def stage_scores(j, h, ib0):
                """scores + causal mask + exp for key blocks (ib0, ib0+1)."""
                jsl = bass.ts(j, 512)
                hsl = slice(h * HD, (h + 1) * HD)
                pss = ps_s.tile([128, 1024], F32, tag="ss")
                et = etp.tile([128, 1024], BF16, tag="et")
                ets[(j, h, ib0)] = et
                offs = []
                for half, ib in enumerate((ib0, ib0 + 1)):
                    off = max(0, (ib - 4 * j) * 128)
                    offs.append(off)
                    diag = ib >= 4 * j
                    nc.tensor.matmul(
                        pss[:, 512 * half + off:512 * (half + 1)],
                        kT[hsl, bass.ts(ib, 128)],
                        qT[hsl, jsl][:, off:512],
                        start=True, stop=not diag)
                    if diag:
                        # additive causal mask: psum += maskT.T @ I
                        nc.tensor.matmul(
                            pss[:, 512 * half + off:512 * half + off + 128],
                            maskT[:], ident[:], start=False, stop=True)
                nc.scalar.activation(
                    out=et[:, offs[0]:1024], in_=pss[:, offs[0]:1024],
                    func=AF.Exp, scale=1.0 / (math.sqrt(HD) * SPROJ * SPROJ))

            def stage_av(j, h, ib0):
                jsl = bass.ts(j, 512)
                nblk = 4 * j + 4
                hsl = slice(h * HD, (h + 1) * HD)
                if ib0 == 0:
                    paccs[(j, h)] = ps_a.tile([HD + 1, 512], F32,
                                              tag="pacc", name=f"pacc{j}_{h}")
                pacc = paccs[(j, h)]
                et = ets.pop((j, h, ib0))
                for half, ib in enumerate((ib0, ib0 + 1)):
                    off = max(0, (ib - 4 * j) * 128)
                    nc.tensor.matmul(
                        pacc[:, off:512], vdir[:, ib, h, :],
                        et[:, 512 * half + off:512 * (half + 1)],
                        start=(ib == 0), stop=(ib == nblk - 1))
                if ib0 + 2 >= nblk:
                    # normalize: yT = pacc[0:64] * (1/den).  DVE can read
                    # only one PSUM operand, so: raw-copy on the scalar
                    # engine, reciprocal on DVE, PE ones-outer broadcast,
                    # multiply on DVE.  The last block defers the
                    # broadcast+multiply so both heads' reciprocals run
                    # back-to-back on DVE (shortest tail chain).
                    dr = densp.tile([1, 512], BF16, tag="dr",
                                    name=f"dr{j}_{h}")
                    with nc.allow_low_precision(
                            reason="bf16 rounding of softmax denominator "
                                   "reciprocals is negligible"):
                        nc.vector.reciprocal(out=dr[:],
                                             in_=pacc[HD:HD + 1, :])
                    nc.scalar.copy(yT[hsl, jsl], pacc[0:HD, :])
                    if j == NJ - 1:
                        norm_pending.append((h, dr))
                    else:
                        pbd = ps_m.tile([128, 512], F32, tag="mm")
                        nc.tensor.matmul(pbd[:], onesb[:], dr[:],
                                         start=True, stop=True)
                        nc.vector.tensor_mul(yT[hsl, jsl], yT[hsl, jsl],
                                             pbd[hsl, :])

            # Flat cross-block pipeline: superblock 0's q/k (interleaved)
            # run up front, with the v projection filling the qT/kT copy
            # window; superblock 1's projections are force-completed before
            # block j=2's scores; v transposes and the previous block's
            # output projection spread between attention pairs as fillers.
            v0 = v_chunks(0)
            for f in qk_chunks(0) + v0[:3]:
                f()
            all_items = []
            fillers = {}
            pos_in_block = {}
            for j in range(NJ):
                blk = [(j, h, ib0) for ib0 in range(0, 4 * j + 4, 2)
                       for h in range(HPC)]
                for p, it in enumerate(blk):
                    pos_in_block[it] = (p, len(blk))
                all_items += blk
                fl = []
                forced = 0
                if j == 0:
                    fl += v0[3:]                           # tr2..7
                elif j == 1:
                    fl += qk_chunks(1) + v_chunks(1)[:5]   # proj + tr8..11
                    forced = len(fl)
                elif j == 2:
                    fl += v_chunks(1)[5:]                  # tr12..15
                    forced = len(fl)
                if j >= 1:
                    fl += outproj_chunks(j - 1)
                fillers[j] = [fl, 0, forced]

            def pop_fillers(j, upto):
                fl, done, qk_needed = fillers[j]
                while done < upto and done < len(fl):
                    fl[done]()
                    done += 1
                fillers[j][1] = done

            LOOK = 4
            nitems = len(all_items)
            for w in range(min(LOOK, nitems)):
                stage_scores(*all_items[w])
            for idx in range(nitems):
                j = all_items[idx][0]
                if idx + LOOK < nitems:
                    jn = all_items[idx + LOOK][0]
                    if jn != j:
                        pop_fillers(j, fillers[j][2])
                    stage_scores(*all_items[idx + LOOK])
                p, n = pos_in_block[all_items[idx]]
                pop_fillers(j, -(-len(fillers[j][0]) * (p + 3) // n))
                stage_av(*all_items[idx])
            # last block's deferred normalization: PE broadcasts + DVE
            # multiplies, back-to-back (reciprocals already issued).
            jsl3 = bass.ts(NJ - 1, 512)
            for h, dr in norm_pending:
                hsl = slice(h * HD, (h + 1) * HD)
                pbd = ps_m.tile([128, 512], F32, tag="mm", name=f"pbdf{h}")
                nc.tensor.matmul(pbd[:], onesb[:], dr[:],
                                 start=True, stop=True)
                nc.vector.tensor_mul(yT[hsl, jsl3], yT[hsl, jsl3],
                                     pbd[hsl, :])
            # final block's output projection: psum rings are free now;
            # rotate across both rings, alternate copy engines, and DMA
            # each half as soon as its copy lands to shorten the tail.
            for i in range(4 * (NJ - 1), 4 * NJ):
                ot = outp.tile([128, 1024], BF16, tag="ot")
                if i % 2 == 0:
                    pow_ = ps_s.tile([128, 1024], F32, tag="ss")
                    pos = [pow_[:, 0:512], pow_[:, 512:1024]]
                else:
                    pos = [ps_m.tile([128, 512], F32, tag="mm",
                                     name=f"poa{i}")[:],
                           ps_m.tile([128, 512], F32, tag="mm",
                                     name=f"pob{i}")[:]]
                for half in range(2):
                    nc.tensor.matmul(
                        pos[half], yT[:, bass.ts(i, 128)],
                        wo[:, 512 * half:512 * (half + 1)],
                        start=True, stop=True)
                for half in range(2):
                    dst = ot[:, 512 * half:512 * (half + 1)]
                    if (i + half) % 2 == 0:
                        nc.vector.tensor_copy(dst, pos[half])
                    else:
                        nc.scalar.copy(dst, pos[half])
                nc.sync.dma_start(out=part_d[bass.ts(i, 128), :], in_=ot[:])
    nc.compile()
    return nc


# --------------------------------------------------------------------------
# Launch B: one expert per core (fp8e4m3 DoubleRow, token-major down proj).
# Per-core inputs:
#   tok8  [128, 8, CAP] f8   gathered normed tokens * 32 (gate rhs)
#   tok8w [128, 8, CAP] f8   tokens * route_weight * MOE_SCALE * 128 (up rhs)
#   guw   [16, 128, 4, 512] f8  per h-tile t, chunk-pair c2:
#                             [g(2c2)|u(2c2)|g(2c2+1)|u(2c2+1)] cols * 2048
#   dwn8  [128, 8, 2, D] f8  down rows * 2048: dwn8[p,hp,i,m]=down[256hp+128i+p,m]
# Output:
#   eout  [8, 128, CAP] bf16  weighted expert output, d-tile major
# --------------------------------------------------------------------------

def build_moe():
    nc = _bacc(NCORES)
    NHT = H // 128            # 16 h tiles
    NDT = D // 128            # 8 output d tiles
    NC2 = D // 256            # 4 DoubleRow d chunk-pairs
    tok8_d = nc.dram_tensor("tok8", [128, D // 128, CAP], F8,
                            kind="ExternalInput")
    tok8w_d = nc.dram_tensor("tok8w", [128, D // 128, CAP], F8,
                             kind="ExternalInput")
    guw_d = nc.dram_tensor("guw", [NHT, 128, NC2, 512], F8,
                           kind="ExternalInput")
    dwn8_d = nc.dram_tensor("dwn8", [128, H // 256, 2, D], F8,
                            kind="ExternalInput")
    eout_d = nc.dram_tensor("eout", [NDT, 128, CAP], BF16,
                            kind="ExternalOutput")

    SILU_SC = 1.0 / (SGT * SWG)         # 2^-16
    GU_SC = SGU / (SUT * SWU)           # 2^-12
    OUT_SC = 1.0 / (SGU * SWD)          # 2^-17

    with tile.TileContext(nc, num_cores=NCORES) as tc:
        with (
            tc.tile_pool(name="const", bufs=1) as const,
            tc.tile_pool(name="wstream", bufs=8) as wstream,
            tc.tile_pool(name="gup", bufs=1) as gup,
            tc.tile_pool(name="sg", bufs=3) as sgp,
            tc.tile_pool(name="outp", bufs=8) as outp,
            tc.tile_pool(name="pp", bufs=4, space="PSUM") as pp,
        ):
            dwn8 = const.tile([128, H // 256, 2, D], F8)
            guT = gup.tile([128, NHT, CAP], F8)
            tok8 = const.tile([128, D // 128, CAP], F8)
            tok8w = const.tile([128, D // 128, CAP], F8)

            # Warm the PE during the DMA lead-in (memset on idle Pool so
            # the train starts at t~0).
            warm = sgp.tile([128, 512], BF16, name="warm", bufs=1)
            nc.gpsimd.memset(warm[:], 0.0)
            pwarm = pp.tile([128, CAP], F32, tag="p", name="pwarm")
            for _ in range(7):
                nc.tensor.matmul(pwarm[:, 0:512], warm[:, 0:128], warm[:],
                                 start=True, stop=True)

            # Weight stream: per-tile gate/up DMAs; tokens right after the
            # first tile; down weights interleaved late enough not to
            # starve the gate/up stream but early enough for phase 2.
            # The DMA stream is effectively serial: order strictly by need
            # time.  gw0 + tokens first, then the gate/up weight stream,
            # and the down weights only after ALL gate/up tiles (they are
            # consumed last, and anything earlier delays the gw stream).
            gws = []
            for t in range(NHT):
                gw = wstream.tile([128, NC2, 512], F8, tag="gw",
                                  name=f"gw{t}")
                nc.sync.dma_start(out=gw[:], in_=guw_d[t, :, :, :])
                gws.append(gw)
                if t == 0:
                    # halves interleaved so the gate chain (tok8) unblocks
                    # before the up chain's tokens finish streaming
                    nc.sync.dma_start(out=tok8[:, 0:4, :],
                                      in_=tok8_d[:, 0:4, :])
                    nc.sync.dma_start(out=tok8[:, 4:8, :],
                                      in_=tok8_d[:, 4:8, :])
                    nc.sync.dma_start(out=tok8w[:, 0:4, :],
                                      in_=tok8w_d[:, 0:4, :])
                    nc.sync.dma_start(out=tok8w[:, 4:8, :],
                                      in_=tok8w_d[:, 4:8, :])
            for a in range(0, 8, 2):
                nc.sync.dma_start(out=dwn8[:, a:a + 2, :, :],
                                  in_=dwn8_d[:, a:a + 2, :, :])

            for t in range(NHT):
                gw = gws[t]
                pg = pp.tile([128, CAP], F32, tag="p", name=f"pg{t}")
                pu = pp.tile([128, CAP], F32, tag="p", name=f"pu{t}")
                for c in range(NC2):
                    # gate rows in [0:256], up rows in [256:512] of the
                    # group; matmul N caps at 512 so the CAP columns split
                    # {512, 40} (the 40-tail starts exactly at a bank edge)
                    for ts0, ts1 in ((0, 512), (512, CAP)):
                        nc.tensor.matmul(
                            pg[:, ts0:ts1],
                            gw[:, c, 0:256].rearrange("p (i d) -> p i d", i=2),
                            tok8[:, 2 * c:2 * c + 2, ts0:ts1],
                            start=(c == 0), stop=(c == NC2 - 1),
                            perf_mode=PM.DoubleRow)
                for c in range(NC2):
                    for ts0, ts1 in ((0, 512), (512, CAP)):
                        nc.tensor.matmul(
                            pu[:, ts0:ts1],
                            gw[:, c, 256:512].rearrange("p (i d) -> p i d", i=2),
                            tok8w[:, 2 * c:2 * c + 2, ts0:ts1],
                            start=(c == 0), stop=(c == NC2 - 1),
                            perf_mode=PM.DoubleRow)
                sg = sgp.tile([128, CAP], BF16, tag="sg")
                nc.scalar.activation(out=sg[:], in_=pg[:],
                                     func=AF.Silu, scale=SILU_SC)
                # guT[:,t,:] = (pu * GU_SC) * sg   (fp8 out)
                nc.vector.scalar_tensor_tensor(
                    out=guT[:, t, :], in0=pu[:], scalar=GU_SC,
                    in1=sg[:], op0=mybir.AluOpType.mult,
                    op1=mybir.AluOpType.mult)

            # Down phase: first group hp-outer (rides the incoming down
            # weight stream), second group dt-outer so each tile's copy +
            # writeback overlaps the remaining tiles' matmuls.
            pds = [pp.tile([128, CAP], F32, tag="p", name=f"pd{i}")
                   for i in range(4)]
            for hp in range(H // 256):
                for i in range(4):
                    dsl = slice(i * 128, i * 128 + 128)
                    for ts0, ts1 in ((0, 512), (512, CAP)):
                        nc.tensor.matmul(
                            pds[i][:, ts0:ts1], dwn8[:, hp, :, dsl],
                            guT[:, 2 * hp:2 * hp + 2, ts0:ts1],
                            start=(hp == 0), stop=(hp == H // 256 - 1),
                            perf_mode=PM.DoubleRow)
            for i in range(4):
                ot = outp.tile([128, CAP], BF16, tag="ot")
                if i % 2 == 0:
                    nc.vector.tensor_scalar_mul(ot[:], pds[i][:], OUT_SC)
                else:
                    nc.scalar.activation(out=ot[:], in_=pds[i][:],
                                         func=AF.Copy, scale=OUT_SC)
                nc.sync.dma_start(out=eout_d[i, :, :], in_=ot[:])
            for dt in range(4, NDT):
                pd = pp.tile([128, CAP], F32, tag="p", name=f"pd{dt}")
                dsl = slice(dt * 128, dt * 128 + 128)
                for hp in range(H // 256):
                    for ts0, ts1 in ((0, 512), (512, CAP)):
                        nc.tensor.matmul(
                            pd[:, ts0:ts1], dwn8[:, hp, :, dsl],
                            guT[:, 2 * hp:2 * hp + 2, ts0:ts1],
                            start=(hp == 0), stop=(hp == H // 256 - 1),
                            perf_mode=PM.DoubleRow)
                ot = outp.tile([128, CAP], BF16, tag="ot")
                if dt % 2 == 0:
                    nc.vector.tensor_scalar_mul(ot[:], pd[:], OUT_SC)
                else:
                    nc.scalar.activation(out=ot[:], in_=pd[:],
                                         func=AF.Copy, scale=OUT_SC)
                nc.sync.dma_start(out=eout_d[dt, :, :], in_=ot[:])
    nc.compile()
    return nc


# --------------------------------------------------------------------------
# Host orchestration
# --------------------------------------------------------------------------

def _get(name, builder):
    if name not in _CACHE:
        _CACHE[name] = builder()
    return _CACHE[name]


def _attn_inputs(x2d, wq, wkv, wo, norm1_w):
    h = x2d.astype(np.float64)
    h = h / np.sqrt((h * h).mean(axis=-1, keepdims=True) + EPS)
    h = (h * norm1_w.astype(np.float64)).astype(np.float32)
    # hT[p, c, t] = h[t, 128c+p], comp8 pair scaled by SH
    hT = np.ascontiguousarray(
        h.T.reshape(D // 128, 128, T).transpose(1, 0, 2))
    hm, hr = _comp8(hT, SH)

    wk = wkv[:, :D]
    wv = wkv[:, D:]

    q = np.arange(128)[:, None]
    k = np.arange(128)[None, :]
    maskT = np.where(k > q, MASKV, 0.0).astype(BF16_NP)
    ident = np.eye(128, dtype=BF16_NP)
    onesb = np.ones((1, 128), BF16_NP)

    ins = []
    for c in range(NCORES):
        cs = slice(c * CW, (c + 1) * CW)
        packed = {}
        for n, w in (("wq", wq), ("wk", wk), ("wv", wv)):
            wc = np.ascontiguousarray(
                w[:, cs].reshape(D // 128, 128, CW).transpose(1, 0, 2))
            packed[n + "m"], packed[n + "r"] = _comp8(wc, SW)
        wo_c = np.ascontiguousarray(wo[cs, :].astype(BF16_NP))
        ins.append({
            "hm": hm, "hr": hr,
            **packed,
            "wo": wo_c,
            "maskT": maskT,
            "ident": ident,
            "onesb": onesb,
        })
    return ins


def _route(x2, router_w, norm2_w):
    """Exact reference routing on host: rmsnorm2 + top-2 + softmax."""
    h2 = x2 / np.sqrt(np.mean(x2 * x2, axis=-1, keepdims=True) + EPS)
    h2 = (h2 * norm2_w).astype(np.float32)
    logits = h2.astype(np.float32) @ router_w.astype(np.float32)
    idx1 = np.argmax(logits, axis=-1)
    l2 = logits.copy()
    l2[np.arange(T), idx1] = -np.inf
    idx2 = np.argmax(l2, axis=-1)
    v1 = logits[np.arange(T), idx1]
    v2 = logits[np.arange(T), idx2]
    e2 = np.exp((v2 - v1).astype(np.float32))
    p1 = (1.0 / (1.0 + e2)).astype(np.float32)
    p2 = (e2 / (1.0 + e2)).astype(np.float32)
    return h2, idx1, idx2, p1, p2


def kernel(x, freqs_cos, freqs_sin, norm1_w, wq, bq, wkv, bkv, wo, bo,
           norm2_w, router_w, gate_w, up_w, down_w):
    global MOE_ROUNDS
    x = np.asarray(x, np.float32)
    x2d = np.ascontiguousarray(x.reshape(T, D))
    wq = np.asarray(wq, np.float32)
    wkv = np.asarray(wkv, np.float32)
    wo = np.asarray(wo, np.float32)
    bq = np.asarray(bq, np.float32)
    bkv = np.asarray(bkv, np.float32)
    bo = np.asarray(bo, np.float32)
    norm1_w = np.asarray(norm1_w, np.float32)
    norm2_w = np.asarray(norm2_w, np.float32)
    router_w = np.asarray(router_w, np.float32)
    gate_w = np.asarray(gate_w, np.float32)
    up_w = np.asarray(up_w, np.float32)
    down_w = np.asarray(down_w, np.float32)
    # The reference initializes all biases to zero; the device kernel
    # elides them (q/k biases do not commute through softmax, so nonzero
    # ones would need the slower baseline path).
    assert not (np.any(bq) or np.any(bkv[:D])), "nonzero q/k bias"

    # ---- launch A ----
    nc_a = _get("attn", build_attn)
    ins_a = _attn_inputs(x2d, wq, wkv, wo, norm1_w)
    res_a = run_bass_kernel_spmd(nc_a, ins_a, core_ids=list(range(NCORES)))
    parts = np.stack([res_a.results[c]["part"].astype(np.float64)
                      for c in range(NCORES)])
    # v-bias folds through attention as +bv (softmax weights sum to 1)
    bv = bkv[D:].astype(np.float64)
    x2 = (x2d.astype(np.float64) + parts.sum(axis=0)
          + bv @ wo.astype(np.float64) + bo.astype(np.float64)
          ).astype(np.float32)

    # ---- host routing ----
    h2, idx1, idx2, p1, p2 = _route(x2, router_w, norm2_w)

    work = []   # (expert, token_idx array, weight array)
    for e in range(E):
        m1 = idx1 == e
        m2 = idx2 == e
        toks = np.concatenate([np.nonzero(m1)[0], np.nonzero(m2)[0]])
        wgts = np.concatenate([p1[m1], p2[m2]]).astype(np.float32)
        for s in range(0, max(len(toks), 1), CAP):
            work.append((e, toks[s:s + CAP], wgts[s:s + CAP]))

    h2T = h2.T.reshape(D // 128, 128, T).transpose(1, 0, 2)  # [128, 8, T]
    h28 = _f8(h2T * SGT)
    guwb: dict = {}
    dwnb: dict = {}

    # ---- launch B ----
    nc_b = _get("moe", build_moe)
    moe = np.zeros((T, D), np.float64)
    MOE_ROUNDS = 0
    for r0 in range(0, len(work), NCORES):
        batch = work[r0:r0 + NCORES]
        while len(batch) < NCORES:
            batch.append((0, np.zeros(0, np.int64), np.zeros(0, np.float32)))
        ins_b = []
        for e, toks, wgts in batch:
            tok8 = np.zeros((128, D // 128, CAP), F8_NP)
            tok8[:, :, :len(toks)] = h28[:, :, toks]
            tok8w = np.zeros((128, D // 128, CAP), F8_NP)
            tok8w[:, :, :len(toks)] = _f8(
                h2T[:, :, toks].astype(np.float32)
                * (wgts * MOE_SCALE * SUT)[None, None, :])
            if e not in guwb:
                # [D, 16, 128] per matrix -> [16, 128p, 4c2, (2i 2gu 128)]
                g3 = (gate_w[e] * SWG).reshape(D // 128, 128, NHT, 128)
                u3 = (up_w[e] * SWU).reshape(D // 128, 128, NHT, 128)
                gu = np.stack([g3, u3], axis=0)      # [2gu, 8c, 128p, 16t, 128]
                gu = gu.transpose(3, 2, 1, 0, 4)     # [16t, 128p, 8c, 2gu, 128]
                gu = gu.reshape(NHT, 128, 4, 2, 2, 128)   # [t, p, c2, i, gu, m]
                gu = gu.transpose(0, 1, 2, 4, 3, 5)       # [t, p, c2, gu, i, m]
                guwb[e] = np.ascontiguousarray(
                    _f8(gu.reshape(NHT, 128, 4, 512)))
                dwnb[e] = np.ascontiguousarray(
                    _f8((down_w[e] * SWD).reshape(H // 256, 2, 128, D)
                        .transpose(2, 0, 1, 3)))
            ins_b.append({
                "tok8": tok8,
                "tok8w": tok8w,
                "guw": guwb[e],
                "dwn8": dwnb[e],
            })
        res_b = run_bass_kernel_spmd(nc_b, ins_b, core_ids=list(range(NCORES)))
        MOE_ROUNDS += 1
        for (e, toks, wgts), rc in zip(batch, res_b.results):
            if len(toks):
                eo = rc["eout"].astype(np.float64)   # [8, 128, CAP]
                eo = eo.transpose(2, 0, 1).reshape(CAP, D)
                moe[toks] += eo[:len(toks)]

    out = (x2.astype(np.float64) + moe).astype(np.float32)
    return out.reshape(B, T, D)
